# revision 5
# baseline (speedup 1.0000x reference)
"""Trainium2 Bass kernel for nn_BlocksCore (RIMs BlocksCore step).

Data-parallel over batch B=2048 across 8 NeuronCores (256 rows each),
parameters replicated. Per-core plan (v2):

  A. input attention in f32 (mask-exact): s1 = (hx_n@wq_n)·(inp@wk1)/8,
     sigmoid collapse of the 2-slot softmax, top-4 mask via rank counts,
     inp_flat = sig*(inp@wv1); transposed to feature-major via DMA-transpose
     (bf16) + fp8 cast copies.
  B. LSTM gates split by precision: i,o gates via fp8e4 DoubleRow matmuls
     (2 k-tiles per instruction), f,g gates via bf16 — per 256-wide hidden
     group (== one attention block) with a double-buffered 2x2-bank PSUM
     pool; elementwise tail + cx masking pipeline per group; h_new^T via
     DMA transpose. Weight columns host-permuted into per-group [i|o]
     (fp8) and [f|g] (bf16) 512-col panels.
  C. communication attention: q/k/v projections, single 32-row score tile
     for all 4 heads x 8 query blocks, one softmax, PE-expanded attention
     apply, gated-residual output and masked merge per block.

Outputs: hx_out/cx_out [256,2048] f32, mask_out [256,8] (host expands).
"""

import json
import os

import numpy as np
import ml_dtypes

BF16 = ml_dtypes.bfloat16
E4 = ml_dtypes.float8_e4m3

B = 2048
NCORES = 8
BSH = B // NCORES          # 256 batch rows per core
NINP = 1024
NHID = 2048
NB = 8                     # blocks
BS = 256                   # block size (NHID / NB)
DKI = 64                   # input-attention d_k

_CACHE = {}
last_exec_time_ns = None
last_results = None

# ---------------------------------------------------------------------------
# BIR post-fix: this toolchain's core_v3 codegen supports only one sync-wait
# per CTRL-class instruction; hoist extras onto single-wait EventSemaphores.
# ---------------------------------------------------------------------------


def _fix_bir_json(bir_bytes: bytes) -> bytes:
    bir = json.loads(bir_bytes)
    for fn in bir.get("functions", []):
        for blk in fn.get("blocks", []):
            out = []
            for ins in blk.get("instructions", []):
                si = ins.get("sync_info") or {}
                waits = si.get("on_wait") or []
                if len(waits) > 1:
                    for j, w in enumerate(waits[:-1]):
                        out.append({
                            "name": f"{ins['name']}-w{j}",
                            "engine": ins["engine"],
                            "opcode": "EventSemaphore",
                            "ins": [],
                            "outs": [],
                            "sync_info": {"on_update": [], "on_wait": [w]},
                        })
                    si = dict(si)
                    si["on_wait"] = [waits[-1]]
                    ins = dict(ins)
                    ins["sync_info"] = si
                out.append(ins)
            blk["instructions"] = out
    return json.dumps(bir).encode()


def _install_bir_fix(nc):
    orig = nc.to_json_bytes

    def patched(*a, **k):
        return _fix_bir_json(orig(*a, **k))

    nc.to_json_bytes = patched


# ---------------------------------------------------------------------------
# Device kernel
# ---------------------------------------------------------------------------

def _build():
    import concourse.bass as bass
    import concourse.tile as tile
    from concourse import mybir

    f32 = mybir.dt.float32
    bf16 = mybir.dt.bfloat16
    fp8 = mybir.dt.float8e4
    OP = mybir.AluOpType
    AF = mybir.ActivationFunctionType
    AX = mybir.AxisListType
    DR = mybir.MatmulPerfMode.DoubleRow

    nc = bass.Bass()

    # ---- I/O ------------------------------------------------------------
    inpT = nc.declare_dram_parameter("inpT", [128, 8, BSH], f32, isOutput=False)
    hxT_f = nc.declare_dram_parameter("hxT_f", [128, 16, BSH], f32, isOutput=False)
    hxT_b = nc.declare_dram_parameter("hxT_b", [128, 16, BSH], bf16, isOutput=False)
    hxT_8 = nc.declare_dram_parameter("hxT_8", [128, 16, BSH], fp8, isOutput=False)
    hx_bm = nc.declare_dram_parameter("hx_bm", [BSH, NHID], f32, isOutput=False)
    cx_bm = nc.declare_dram_parameter("cx_bm", [BSH, NHID], f32, isOutput=False)
    wq = nc.declare_dram_parameter("wq", [128, 2, NB, DKI], f32, isOutput=False)
    wk1 = nc.declare_dram_parameter("wk1", [128, 8, DKI], f32, isOutput=False)
    wv1 = nc.declare_dram_parameter("wv1", [128, 8, BS], f32, isOutput=False)
    # LSTM weights: [128, 32 ktiles, 8 groups * 512] — per group g the fp8
    # panel holds [i|o] columns for hidden chunk g, the bf16 panel [f|g].
    w8d = nc.declare_dram_parameter("w8d", [128, 32, 4096], fp8, isOutput=False)
    wbfd = nc.declare_dram_parameter("wbfd", [128, 32, 4096], bf16, isOutput=False)
    bias8 = nc.declare_dram_parameter("bias8", [1, 4096], bf16, isOutput=False)
    biasbf = nc.declare_dram_parameter("biasbf", [1, 4096], bf16, isOutput=False)
    wqc = nc.declare_dram_parameter("wqc", [128, 2, NB, 128], bf16, isOutput=False)
    wkc = nc.declare_dram_parameter("wkc", [128, 2, NB, 128], bf16, isOutput=False)
    wvc = nc.declare_dram_parameter("wvc", [128, 2, NB, 128], bf16, isOutput=False)
    fgw = nc.declare_dram_parameter("fgw", [128, 2 * BS], bf16, isOutput=False)
    fgb = nc.declare_dram_parameter("fgb", [1, 2 * BS], bf16, isOutput=False)
    hx_out = nc.declare_dram_parameter("hx_out", [BSH, NHID], f32, isOutput=True)
    cx_out = nc.declare_dram_parameter("cx_out", [BSH, NHID], f32, isOutput=True)
    mask_out = nc.declare_dram_parameter("mask_out", [BSH, NB], f32, isOutput=True)

    # ---- inline constants ----------------------------------------------
    # score-placement selector: head h of a 128-feature product vector sums
    # into row h*8+q  (d -> h = d//32)
    hq_np = np.zeros((128, NB, 32), dtype=BF16)
    for d in range(128):
        for q in range(NB):
            hq_np[d, q, (d // 32) * 8 + q] = 1
    # head expander: out feature m (=h*32+d) reads score row (m//32)*8 + q
    e32_np = np.zeros((32, NB, 128), dtype=BF16)
    for m in range(128):
        for q in range(NB):
            e32_np[(m // 32) * 8 + q, q, m] = 1
    hqc = nc.inline_tensor(hq_np, "hqc")
    e32b = nc.inline_tensor(e32_np, "e32b")
    ones1c = nc.inline_tensor(np.ones((1, 128), dtype=BF16), "ones1c")

    with tile.TileContext(nc) as tc:
        with tc.tile_pool(name="cp", bufs=1) as cp, \
             tc.tile_pool(name="pp", bufs=1) as pp:
            # constants to SBUF (gpsimd queue keeps sync/scalar free)
            hq_sb = cp.tile([128, NB, 32], bf16)
            nc.gpsimd.dma_start(out=hq_sb[:], in_=hqc[:])
            e32_sb = cp.tile([32, NB, 128], bf16)
            nc.gpsimd.dma_start(out=e32_sb[:], in_=e32b[:])
            ones1_sb = cp.tile([1, 128], bf16)
            nc.gpsimd.dma_start(out=ones1_sb[:], in_=ones1c[:])
            fgw_sb = cp.tile([128, 2 * BS], bf16)
            nc.gpsimd.dma_start(out=fgw_sb[:], in_=fgw[:])
            fgb_sb = cp.tile([1, 2 * BS], bf16)
            nc.gpsimd.dma_start(out=fgb_sb[:], in_=fgb[:])
            wqc_sb = cp.tile([128, 2, NB, 128], bf16)
            nc.gpsimd.dma_start(out=wqc_sb[:], in_=wqc[:])
            wkc_sb = cp.tile([128, 2, NB, 128], bf16)
            nc.gpsimd.dma_start(out=wkc_sb[:], in_=wkc[:])
            wvc_sb = cp.tile([128, 2, NB, 128], bf16)
            nc.gpsimd.dma_start(out=wvc_sb[:], in_=wvc[:])
            bias8_sb = cp.tile([1, 4096], bf16)
            nc.sync.dma_start(out=bias8_sb[:], in_=bias8[:])
            biasbf_sb = cp.tile([1, 4096], bf16)
            nc.scalar.dma_start(out=biasbf_sb[:], in_=biasbf[:])

            # LSTM activations (feature-major)
            hxT8_sb = pp.tile([128, 16, BSH], fp8)
            nc.sync.dma_start(out=hxT8_sb[:], in_=hxT_8[:])
            hxTb_sb = pp.tile([128, 16, BSH], bf16)
            nc.sync.dma_start(out=hxTb_sb[:], in_=hxT_b[:])
            cx_sb = [pp.tile([128, NHID], f32, tag=f"cx{bt}", name=f"cx{bt}")
                     for bt in range(2)]
            for bt in range(2):
                nc.gpsimd.dma_start(out=cx_sb[bt][:],
                                    in_=cx_bm[bt * 128:(bt + 1) * 128, :])
            hx_sb = [pp.tile([128, NHID], f32, tag=f"hx{bt}", name=f"hx{bt}")
                     for bt in range(2)]
            for bt in range(2):
                nc.gpsimd.dma_start(out=hx_sb[bt][:],
                                    in_=hx_bm[bt * 128:(bt + 1) * 128, :])

            xt8_sb = pp.tile([128, 16, 2, 128], fp8)
            xtb_sb = pp.tile([128, 16, 2, 128], bf16)
            hnew_sb = [pp.tile([128, NHID], f32, tag=f"hn{bt}", name=f"hn{bt}")
                       for bt in range(2)]
            hnewT_sb = pp.tile([128, 16, BSH], bf16)
            mask_sb = [pp.tile([128, NB], f32, tag=f"mk{bt}", name=f"mk{bt}")
                       for bt in range(2)]
            sig_sb = [pp.tile([128, NB], f32, tag=f"sg{bt}", name=f"sg{bt}")
                      for bt in range(2)]
            qc_sb = pp.tile([128, NB, BSH], bf16)
            kc_sb = pp.tile([128, NB, BSH], bf16)
            vc_sb = pp.tile([128, NB, BSH], bf16)

            # ---- phase A (f32, mask-exact) -------------------------------
            with tc.tile_pool(name="pa", bufs=1) as pa, \
                 tc.tile_pool(name="pa2", bufs=2) as pa2, \
                 tc.tile_pool(name="paps", bufs=2, space="PSUM") as paps:
                inpT_sb = pa.tile([128, 8, BSH], f32)
                nc.gpsimd.dma_start(out=inpT_sb[:], in_=inpT[:])
                wk1_sb = pa.tile([128, 8, DKI], f32)
                nc.gpsimd.dma_start(out=wk1_sb[:], in_=wk1[:])
                wv1_sb = pa.tile([128, 8, BS], f32)
                nc.gpsimd.dma_start(out=wv1_sb[:], in_=wv1[:])
                wq_sb = pa.tile([128, 2, NB, DKI], f32)
                nc.gpsimd.dma_start(out=wq_sb[:], in_=wq[:])
                hxTf_sb = pa.tile([128, 16, BSH], f32)
                nc.gpsimd.dma_start(out=hxTf_sb[:], in_=hxT_f[:])

                for bt in range(2):
                    bsl = slice(bt * 128, (bt + 1) * 128)
                    k1_ps = paps.tile([128, DKI], f32, tag="k1")
                    for k in range(8):
                        nc.tensor.matmul(k1_ps[:], inpT_sb[:, k, bsl],
                                         wk1_sb[:, k, :],
                                         start=(k == 0), stop=(k == 7))
                    k1s = pa2.tile([128, DKI], f32, tag="k1s")
                    nc.vector.tensor_copy(k1s[:], k1_ps[:])

                    v1_ps = paps.tile([128, BS], f32, tag="v1")
                    for k in range(8):
                        nc.tensor.matmul(v1_ps[:], inpT_sb[:, k, bsl],
                                         wv1_sb[:, k, :],
                                         start=(k == 0), stop=(k == 7))
                    v1s = pa2.tile([128, BS], f32, tag="v1s")
                    nc.vector.tensor_copy(v1s[:], v1_ps[:])

                    q_ps = paps.tile([128, NB, DKI], f32, tag="q")
                    for n in range(NB):
                        for s in range(2):
                            nc.tensor.matmul(q_ps[:, n, :],
                                             hxTf_sb[:, 2 * n + s, bsl],
                                             wq_sb[:, s, n, :],
                                             start=(s == 0), stop=(s == 1))
                    prod = pa2.tile([128, NB, DKI], f32, tag="prod")
                    k1a = k1s[:]
                    k1bc = bass.AP(tensor=k1a.tensor, offset=k1a.offset,
                                   ap=[k1a.ap[0], [0, NB], k1a.ap[1]])
                    nc.vector.tensor_tensor(prod[:], q_ps[:], k1bc, OP.mult)
                    s1 = pa2.tile([128, NB], f32, tag="s1")
                    nc.vector.reduce_sum(s1[:], prod[:], axis=AX.X)
                    nc.scalar.activation(sig_sb[bt][:], s1[:], AF.Sigmoid,
                                         scale=0.125)

                    # top-4 mask: keep blocks whose s1 is among the 4 largest
                    cnt = pa2.tile([128, NB], f32, tag="cnt")
                    tmp = pa2.tile([128, NB], f32, tag="tmp")
                    for n in range(NB):
                        nc.vector.tensor_single_scalar(tmp[:], s1[:],
                                                       s1[:, n:n + 1], OP.is_gt)
                        nc.vector.reduce_sum(cnt[:, n:n + 1], tmp[:], axis=AX.X)
                    nc.vector.tensor_single_scalar(mask_sb[bt][:], cnt[:], 4.0,
                                                   OP.is_lt)
                    nc.gpsimd.dma_start(out=mask_out[bsl, :], in_=mask_sb[bt][:])

                    # inp_flat (batch-major bf16) -> feature-major via DMA
                    # transpose; fp8 cast copies for the DoubleRow path
                    ifl = pa2.tile([128, NB, BS], bf16, tag="ifl")
                    for n in range(NB):
                        nc.vector.tensor_single_scalar(ifl[:, n, :], v1s[:],
                                                       sig_sb[bt][:, n:n + 1],
                                                       OP.mult)
                    for ft in range(16):
                        nc.scalar.dma_start(
                            out=xtb_sb[:, ft, bt, :],
                            in_=ifl[:, ft // 2, (ft % 2) * 128:(ft % 2) * 128 + 128],
                            transpose=True)
                        nc.vector.tensor_copy(xt8_sb[:, ft, bt, :],
                                              xtb_sb[:, ft, bt, :])

            # ---- phase B: LSTM groups ------------------------------------
            # pair order: hh first (j=8..15) so PE never waits on phase A
            pair_order = list(range(8, 16)) + list(range(8))
            with tc.tile_pool(name="gps", bufs=2, space="PSUM") as gps, \
                 tc.tile_pool(name="pw", bufs=6) as pw, \
                 tc.tile_pool(name="pb2", bufs=2) as pb2:
                for gq in range(8):
                    g_all = [gps.tile([128, 4, BS], f32, tag=f"g{bt}",
                                      name=f"g{bt}") for bt in range(2)]
                    csl = slice(gq * 512, (gq + 1) * 512)
                    for bt in range(2):
                        nc.tensor.matmul(g_all[bt][:, 0:2, :], ones1_sb[:],
                                         bias8_sb[:, csl], start=True, stop=False)
                        nc.tensor.matmul(g_all[bt][:, 2:4, :], ones1_sb[:],
                                         biasbf_sb[:, csl], start=True, stop=False)
                    for j in pair_order:
                        w8t = pw.tile([128, 2, 512], fp8, tag="w8t")
                        nc.sync.dma_start(out=w8t[:],
                                          in_=w8d[:, 2 * j:2 * j + 2, csl])
                        wbt = pw.tile([128, 2, 512], bf16, tag="wbt")
                        nc.scalar.dma_start(out=wbt[:],
                                            in_=wbfd[:, 2 * j:2 * j + 2, csl])
                        st = (j == pair_order[-1])
                        for bt in range(2):
                            bsl = slice(bt * 128, (bt + 1) * 128)
                            if j >= 8:
                                t = 2 * (j - 8)
                                lhs8 = hxT8_sb[:, t:t + 2, bsl]
                                lhsb = [hxTb_sb[:, t + kk, bsl] for kk in range(2)]
                            else:
                                lhs8 = xt8_sb[:, 2 * j:2 * j + 2, bt, :]
                                lhsb = [xtb_sb[:, 2 * j + kk, bt, :]
                                        for kk in range(2)]
                            nc.tensor.matmul(g_all[bt][:, 0:2, :], lhs8, w8t[:],
                                             start=False, stop=st, perf_mode=DR)
                            for kk in range(2):
                                nc.tensor.matmul(g_all[bt][:, 2:4, :], lhsb[kk],
                                                 wbt[:, kk, :], start=False,
                                                 stop=(st and kk == 1))
                    # ---- group tail: LSTM elementwise, cx merge, h_new^T
                    sl = slice(gq * BS, (gq + 1) * BS)
                    for bt in range(2):
                        sio = pb2.tile([128, 2, BS], f32, tag="sio", name="sio")
                        nc.scalar.activation(sio[:], g_all[bt][:, 0:2, :],
                                             AF.Sigmoid)
                        sigf = pb2.tile([128, BS], f32, tag="sigf", name="sigf")
                        nc.scalar.activation(sigf[:], g_all[bt][:, 2, :],
                                             AF.Sigmoid)
                        tang = pb2.tile([128, BS], f32, tag="tang", name="tang")
                        nc.scalar.activation(tang[:], g_all[bt][:, 3, :], AF.Tanh)
                        t1 = pb2.tile([128, BS], f32, tag="t1", name="t1")
                        nc.vector.tensor_tensor(t1[:], sigf[:], cx_sb[bt][:, sl],
                                                OP.mult)
                        t2 = pb2.tile([128, BS], f32, tag="t2", name="t2")
                        nc.vector.tensor_tensor(t2[:], sio[:, 0, :], tang[:],
                                                OP.mult)
                        cnew = pb2.tile([128, BS], f32, tag="cnew", name="cnew")
                        nc.vector.tensor_tensor(cnew[:], t1[:], t2[:], OP.add)
                        t3 = pb2.tile([128, BS], f32, tag="t3", name="t3")
                        nc.scalar.activation(t3[:], cnew[:], AF.Tanh)
                        nc.vector.tensor_tensor(hnew_sb[bt][:, sl], sio[:, 1, :],
                                                t3[:], OP.mult)
                        hnb = pb2.tile([128, BS], bf16, tag="hnb", name="hnb")
                        nc.scalar.copy(hnb[:], hnew_sb[bt][:, sl])
                        # cx merge + output now (overlaps rest of B)
                        dc = pb2.tile([128, BS], f32, tag="dc", name="dc")
                        nc.vector.tensor_tensor(dc[:], cnew[:], cx_sb[bt][:, sl],
                                                OP.subtract)
                        co = pb2.tile([128, BS], f32, tag="co", name="co")
                        nc.vector.scalar_tensor_tensor(co[:], dc[:],
                                                       mask_sb[bt][:, gq:gq + 1],
                                                       cx_sb[bt][:, sl],
                                                       OP.mult, OP.add)
                        nc.gpsimd.dma_start(
                            out=cx_out[bt * 128:(bt + 1) * 128, sl], in_=co[:])
                        # feature-major h_new via DMA transpose (bf16)
                        for s in range(2):
                            nc.scalar.dma_start(
                                out=hnewT_sb[:, 2 * gq + s,
                                             bt * 128:(bt + 1) * 128],
                                in_=hnb[:, s * 128:(s + 1) * 128], transpose=True)

            # ============================ phase C ========================
            with tc.tile_pool(name="pc", bufs=1) as pc, \
                 tc.tile_pool(name="pctmp", bufs=3) as pctmp:
                at_sb = pc.tile([32, NB, BSH], bf16)
                coutb_sb = pc.tile([128, NB, BSH], bf16)
                with tc.tile_pool(name="psS", bufs=1, space="PSUM") as psS, \
                     tc.tile_pool(name="prjC", bufs=2, space="PSUM") as prj:
                    # k/v projections first, then per-q: q-proj -> pr -> S
                    for wsb, dst in ((wkc_sb, kc_sb), (wvc_sb, vc_sb)):
                        for n in range(NB):
                            ps = prj.tile([128, BSH], f32, tag="proj")
                            for s in range(2):
                                nc.tensor.matmul(ps[:], wsb[:, s, n, :],
                                                 hnewT_sb[:, 2 * n + s, :],
                                                 start=(s == 0), stop=(s == 1))
                            nc.scalar.copy(dst[:, n, :], ps[:])
                    S = psS.tile([32, NB, BSH], f32, tag="S", name="S")
                    for q in range(NB):
                        ps = prj.tile([128, BSH], f32, tag="proj")
                        for s in range(2):
                            nc.tensor.matmul(ps[:], wqc_sb[:, s, q, :],
                                             hnewT_sb[:, 2 * q + s, :],
                                             start=(s == 0), stop=(s == 1))
                        nc.scalar.copy(qc_sb[:, q, :], ps[:])
                        pr = pctmp.tile([128, NB, BSH], bf16, tag="pr", name="pr")
                        qa = qc_sb[:, q, :]
                        qbc = bass.AP(tensor=qa.tensor, offset=qa.offset,
                                      ap=[qa.ap[0], [0, NB], qa.ap[-1]])
                        nc.vector.tensor_tensor(pr[:], qbc, kc_sb[:], OP.mult)
                        for kp in range(4):
                            nc.tensor.matmul(S[:, 2 * kp:2 * kp + 2, :],
                                             hq_sb[:, q, :],
                                             pr[:, 2 * kp:2 * kp + 2, :],
                                             start=(q == 0), stop=(q == 7))
                    ex = pc.tile([32, NB, BSH], bf16, tag="ex", name="ex")
                    nc.scalar.activation(ex[:], S[:], AF.Exp,
                                         scale=float(1.0 / np.sqrt(32.0)))
                    denom = pctmp.tile([32, BSH], f32, tag="denom", name="denom")
                    nc.vector.reduce_sum(denom[:],
                                         ex[:].rearrange("p k b -> p b k"),
                                         axis=AX.X)
                    recip = pctmp.tile([32, BSH], f32, tag="recip", name="recip")
                    nc.vector.reciprocal(recip[:], denom[:])
                    ra = recip[:]
                    rbc = bass.AP(tensor=ra.tensor, offset=ra.offset,
                                  ap=[ra.ap[0], [0, NB], ra.ap[-1]])
                    nc.vector.tensor_tensor(at_sb[:], ex[:], rbc, OP.mult)

                with tc.tile_pool(name="psU", bufs=1, space="PSUM") as psU, \
                     tc.tile_pool(name="psOG", bufs=2, space="PSUM") as psOG:
                    for q in range(NB):
                        U = psU.tile([128, NB, BSH], f32, tag="U", name="U")
                        for kp in range(4):
                            nc.tensor.matmul(U[:, 2 * kp:2 * kp + 2, :],
                                             e32_sb[:, q, :],
                                             at_sb[:, 2 * kp:2 * kp + 2, :],
                                             start=True, stop=True)
                        prods = pctmp.tile([128, NB, BSH], bf16, tag="prods")
                        nc.vector.tensor_tensor(prods[:], U[:], vc_sb[:], OP.mult)
                        with nc.allow_low_precision(
                                reason="8-way sum of O(0.1) attn outputs"):
                            nc.vector.reduce_sum(
                                coutb_sb[:, q, :],
                                prods[:].rearrange("p m b -> p b m"),
                                axis=AX.X)
                        # gated residual + masked merge for this block
                        for bt in range(2):
                            csl = coutb_sb[:, q, bt * 128:(bt + 1) * 128]
                            og = psOG.tile([128, 2 * BS], f32, tag="og", name="og")
                            nc.tensor.matmul(og[:], csl, fgw_sb[:],
                                             start=True, stop=False)
                            nc.tensor.matmul(og[:], ones1_sb[:], fgb_sb[:],
                                             start=False, stop=True)
                            tano = pctmp.tile([128, BS], f32, tag="tano",
                                              name="tano")
                            nc.scalar.activation(tano[:], og[:, 0:BS], AF.Tanh)
                            sg = pctmp.tile([128, BS], f32, tag="sgx", name="sgx")
                            nc.scalar.activation(sg[:], og[:, BS:2 * BS],
                                                 AF.Sigmoid)
                            hatt = pctmp.tile([128, BS], f32, tag="hatt",
                                              name="hatt")
                            nc.vector.tensor_tensor(hatt[:], sg[:], tano[:],
                                                    OP.mult)
                            qsl = slice(q * BS, (q + 1) * BS)
                            nc.gpsimd.tensor_tensor(hnew_sb[bt][:, qsl],
                                                    hnew_sb[bt][:, qsl],
                                                    hatt[:], OP.add)
                            dh = pctmp.tile([128, BS], f32, tag="dhq", name="dhq")
                            nc.gpsimd.tensor_tensor(dh[:], hnew_sb[bt][:, qsl],
                                                    hx_sb[bt][:, qsl],
                                                    OP.subtract)
                            ho = pctmp.tile([128, BS], f32, tag="hoq", name="hoq",
                                            bufs=4)
                            nc.vector.scalar_tensor_tensor(ho[:], dh[:],
                                                           mask_sb[bt][:, q:q + 1],
                                                           hx_sb[bt][:, qsl],
                                                           OP.mult, OP.add)
                            nc.gpsimd.dma_start(
                                out=hx_out[bt * 128:(bt + 1) * 128, qsl],
                                in_=ho[:])

    _install_bir_fix(nc)
    return nc


# ---------------------------------------------------------------------------
# Host wrapper
# ---------------------------------------------------------------------------

def kernel(inp, hx, cx, wq_inp, wk_inp, wv_inp, w_ih, w_hh, b_ih, b_hh,
           wq_c, wk_c, wv_c, fc_w, fc_b, gate_w, gate_b, step=None):
    global last_exec_time_ns, last_results

    inp = np.asarray(inp, np.float32)
    hx = np.asarray(hx, np.float32)
    cx = np.asarray(cx, np.float32)
    wq_inp = np.asarray(wq_inp, np.float32)
    wk_inp = np.asarray(wk_inp, np.float32)
    wv_inp = np.asarray(wv_inp, np.float32)
    w_ih = np.asarray(w_ih, np.float32)
    w_hh = np.asarray(w_hh, np.float32)
    b_ih = np.asarray(b_ih, np.float32)
    b_hh = np.asarray(b_hh, np.float32)
    wq_c = np.asarray(wq_c, np.float32)
    wk_c = np.asarray(wk_c, np.float32)
    wv_c = np.asarray(wv_c, np.float32)
    fc_w = np.asarray(fc_w, np.float32)
    fc_b = np.asarray(fc_b, np.float32)
    gate_w = np.asarray(gate_w, np.float32)
    gate_b = np.asarray(gate_b, np.float32)

    if "nc" not in _CACHE:
        _CACHE["nc"] = _build()
    nc = _CACHE["nc"]

    # column permutations: per 256-wide hidden group g the fp8 panel holds
    # [i|o], the bf16 panel [f|g]  (torch gate order i,f,g,o)
    wcat = np.concatenate([w_ih.T, w_hh.T], axis=0)     # (4096, 8192)
    bias = (b_ih + b_hh)
    perm8 = np.concatenate([np.concatenate([
        np.arange(0 * NHID + g * BS, 0 * NHID + (g + 1) * BS),
        np.arange(3 * NHID + g * BS, 3 * NHID + (g + 1) * BS)])
        for g in range(8)])
    permbf = np.concatenate([np.concatenate([
        np.arange(1 * NHID + g * BS, 1 * NHID + (g + 1) * BS),
        np.arange(2 * NHID + g * BS, 2 * NHID + (g + 1) * BS)])
        for g in range(8)])
    w8_np = wcat[:, perm8].astype(E4)                   # (4096, 4096)
    wbf_np = wcat[:, permbf].astype(BF16)
    # ktile-major, partition-first: [128, 32, 4096]
    w8d = np.ascontiguousarray(w8_np.reshape(32, 128, 4096).transpose(1, 0, 2))
    wbfd = np.ascontiguousarray(wbf_np.reshape(32, 128, 4096).transpose(1, 0, 2))

    shared = {
        "wq": np.ascontiguousarray(
            wq_inp.reshape(NB, 2, 128, DKI).transpose(2, 1, 0, 3)),
        "wk1": np.ascontiguousarray(
            wk_inp[1].reshape(8, 128, DKI).transpose(1, 0, 2)),
        "wv1": np.ascontiguousarray(
            wv_inp[1].reshape(8, 128, BS).transpose(1, 0, 2)),
        "w8d": w8d,
        "wbfd": wbfd,
        "bias8": bias[perm8].astype(BF16).reshape(1, 4096),
        "biasbf": bias[permbf].astype(BF16).reshape(1, 4096),
        "wqc": np.ascontiguousarray(
            wq_c.astype(BF16).reshape(NB, 2, 128, 128).transpose(2, 1, 0, 3)),
        "wkc": np.ascontiguousarray(
            wk_c.astype(BF16).reshape(NB, 2, 128, 128).transpose(2, 1, 0, 3)),
        "wvc": np.ascontiguousarray(
            wv_c.astype(BF16).reshape(NB, 2, 128, 128).transpose(2, 1, 0, 3)),
        "fgw": np.ascontiguousarray(
            np.concatenate([fc_w, gate_w], axis=1)).astype(BF16),
        "fgb": np.concatenate([fc_b, gate_b]).astype(BF16).reshape(1, 2 * BS),
    }

    in_maps = []
    for c in range(NCORES):
        rs = slice(c * BSH, (c + 1) * BSH)
        inpT_c = inp[rs].T.reshape(8, 128, BSH).transpose(1, 0, 2)
        hxT = hx[rs].T.reshape(16, 128, BSH).transpose(1, 0, 2)
        m = {
            "inpT": np.ascontiguousarray(inpT_c),
            "hxT_f": np.ascontiguousarray(hxT),
            "hxT_b": np.ascontiguousarray(hxT.astype(BF16)),
            "hxT_8": np.ascontiguousarray(hxT.astype(E4)),
            "hx_bm": np.ascontiguousarray(hx[rs]),
            "cx_bm": np.ascontiguousarray(cx[rs]),
        }
        m.update(shared)
        in_maps.append(m)

    from concourse.bass_utils import run_bass_kernel_spmd
    trace = bool(int(os.environ.get("BASS_KTRACE", "0")))
    res = run_bass_kernel_spmd(nc, in_maps, list(range(NCORES)), trace=trace)
    last_exec_time_ns = res.exec_time_ns
    last_results = res

    hx_full = np.empty((B, NHID), np.float32)
    cx_full = np.empty((B, NHID), np.float32)
    mask_full = np.empty((B, NHID), np.float32)
    for c in range(NCORES):
        rs = slice(c * BSH, (c + 1) * BSH)
        hx_full[rs] = res.results[c]["hx_out"]
        cx_full[rs] = res.results[c]["cx_out"]
        mask_full[rs] = np.repeat(res.results[c]["mask_out"], BS, axis=1)
    return hx_full, cx_full, mask_full


# revision 7
# speedup vs baseline: 1.0087x; 1.0087x over previous
"""Trainium2 Bass kernel for nn_BlocksCore (RIMs BlocksCore step).

Data-parallel over batch B=2048 across 8 NeuronCores (256 rows each),
parameters replicated. Per-core plan (v3):

  A. input attention in f32 (mask-exact); inp_flat transposed to
     feature-major via DMA-transpose (bf16) + fp8 cast copies.
  B. LSTM gates: i,o via fp8e4 DoubleRow matmuls, f,g via bf16, processed
     per 256-wide hidden group (== one attention block); weight panels
     fetched two groups at a time ([128,2,1024] tiles) on the sync (fp8)
     and scalar (bf16) HWDGE queues; group tails (activations, c/h update,
     cx merge, h_new^T DMA-transpose) pipeline under the next group's
     matmuls. d0 = h_new - hx precomputed for the final merge.
  C. communication attention: q/k/v projections, one 32-row score tile for
     all (head, q-block) pairs, single softmax, PE-expanded apply with
     bf16 tree reductions, gated residual + masked merge per block.

Outputs: hx_out/cx_out [256,2048] f32, mask_out [256,8] (host expands).
"""

import json
import os

import numpy as np
import ml_dtypes

BF16 = ml_dtypes.bfloat16
E4 = ml_dtypes.float8_e4m3

B = 2048
NCORES = 8
BSH = B // NCORES          # 256 batch rows per core
NINP = 1024
NHID = 2048
NB = 8                     # blocks
BS = 256                   # block size (NHID / NB)
DKI = 64                   # input-attention d_k

_CACHE = {}
last_exec_time_ns = None
last_results = None

# ---------------------------------------------------------------------------
# BIR post-fix: this toolchain's core_v3 codegen supports only one sync-wait
# per CTRL-class instruction; hoist extras onto single-wait EventSemaphores.
# ---------------------------------------------------------------------------


def _fix_bir_json(bir_bytes: bytes) -> bytes:
    bir = json.loads(bir_bytes)
    for fn in bir.get("functions", []):
        for blk in fn.get("blocks", []):
            out = []
            for ins in blk.get("instructions", []):
                si = ins.get("sync_info") or {}
                waits = si.get("on_wait") or []
                if len(waits) > 1:
                    for j, w in enumerate(waits[:-1]):
                        out.append({
                            "name": f"{ins['name']}-w{j}",
                            "engine": ins["engine"],
                            "opcode": "EventSemaphore",
                            "ins": [],
                            "outs": [],
                            "sync_info": {"on_update": [], "on_wait": [w]},
                        })
                    si = dict(si)
                    si["on_wait"] = [waits[-1]]
                    ins = dict(ins)
                    ins["sync_info"] = si
                out.append(ins)
            blk["instructions"] = out
    return json.dumps(bir).encode()


def _install_bir_fix(nc):
    orig = nc.to_json_bytes

    def patched(*a, **k):
        return _fix_bir_json(orig(*a, **k))

    nc.to_json_bytes = patched


# ---------------------------------------------------------------------------
# Device kernel
# ---------------------------------------------------------------------------

def _build():
    import concourse.bass as bass
    import concourse.tile as tile
    from concourse import mybir

    f32 = mybir.dt.float32
    bf16 = mybir.dt.bfloat16
    fp8 = mybir.dt.float8e4
    OP = mybir.AluOpType
    AF = mybir.ActivationFunctionType
    AX = mybir.AxisListType
    DR = mybir.MatmulPerfMode.DoubleRow

    nc = bass.Bass()

    # ---- I/O ------------------------------------------------------------
    inpT = nc.declare_dram_parameter("inpT", [128, 8, BSH], f32, isOutput=False)
    hxT_f = nc.declare_dram_parameter("hxT_f", [128, 16, BSH], f32, isOutput=False)
    hxT_b = nc.declare_dram_parameter("hxT_b", [128, 16, BSH], bf16, isOutput=False)
    hxT_8 = nc.declare_dram_parameter("hxT_8", [128, 16, BSH], fp8, isOutput=False)
    hx_bm = nc.declare_dram_parameter("hx_bm", [BSH, NHID], f32, isOutput=False)
    cx_bm = nc.declare_dram_parameter("cx_bm", [BSH, NHID], f32, isOutput=False)
    wq = nc.declare_dram_parameter("wq", [128, 2, NB, DKI], f32, isOutput=False)
    wk1 = nc.declare_dram_parameter("wk1", [128, 8, DKI], f32, isOutput=False)
    wv1 = nc.declare_dram_parameter("wv1", [128, 8, BS], f32, isOutput=False)
    # LSTM weights: [128, 32 ktiles, 8 groups * 512] — per group g the fp8
    # panel holds [i|o] columns for hidden chunk g, the bf16 panel [f|g].
    w8d = nc.declare_dram_parameter("w8d", [128, 32, 4096], fp8, isOutput=False)
    wbfd = nc.declare_dram_parameter("wbfd", [128, 32, 4096], bf16, isOutput=False)
    bias8 = nc.declare_dram_parameter("bias8", [1, 4096], bf16, isOutput=False)
    biasbf = nc.declare_dram_parameter("biasbf", [1, 4096], bf16, isOutput=False)
    wqc = nc.declare_dram_parameter("wqc", [128, 2, NB, 128], bf16, isOutput=False)
    wkc = nc.declare_dram_parameter("wkc", [128, 2, NB, 128], bf16, isOutput=False)
    wvc = nc.declare_dram_parameter("wvc", [128, 2, NB, 128], bf16, isOutput=False)
    fgw = nc.declare_dram_parameter("fgw", [128, 2 * BS], bf16, isOutput=False)
    fgb = nc.declare_dram_parameter("fgb", [1, 2 * BS], bf16, isOutput=False)
    hx_out = nc.declare_dram_parameter("hx_out", [BSH, NHID], f32, isOutput=True)
    cx_out = nc.declare_dram_parameter("cx_out", [BSH, NHID], f32, isOutput=True)
    mask_out = nc.declare_dram_parameter("mask_out", [BSH, NB], f32, isOutput=True)

    # ---- inline constants ----------------------------------------------
    hq_np = np.zeros((128, NB, 32), dtype=BF16)
    for d in range(128):
        for q in range(NB):
            hq_np[d, q, (d // 32) * 8 + q] = 1
    e32_np = np.zeros((32, NB, 128), dtype=BF16)
    for m in range(128):
        for q in range(NB):
            e32_np[(m // 32) * 8 + q, q, m] = 1
    hqc = nc.inline_tensor(hq_np, "hqc")
    e32b = nc.inline_tensor(e32_np, "e32b")
    ones1c = nc.inline_tensor(np.ones((1, 128), dtype=BF16), "ones1c")

    with tile.TileContext(nc) as tc:
        with tc.tile_pool(name="cp", bufs=1) as cp, \
             tc.tile_pool(name="pp", bufs=1) as pp:
            # fast-path inputs on sync (needed within ~5us)
            bias8_sb = cp.tile([1, 4096], bf16)
            nc.sync.dma_start(out=bias8_sb[:], in_=bias8[:])
            hxT8_sb = pp.tile([128, 16, BSH], fp8)
            nc.sync.dma_start(out=hxT8_sb[:], in_=hxT_8[:])
            hxTb_sb = pp.tile([128, 16, BSH], bf16)
            nc.sync.dma_start(out=hxTb_sb[:], in_=hxT_b[:])

            # constants and late inputs on gpsimd
            hq_sb = cp.tile([128, NB, 32], bf16)
            nc.gpsimd.dma_start(out=hq_sb[:], in_=hqc[:])
            e32_sb = cp.tile([32, NB, 128], bf16)
            nc.gpsimd.dma_start(out=e32_sb[:], in_=e32b[:])
            ones1_sb = cp.tile([1, 128], bf16)
            nc.gpsimd.dma_start(out=ones1_sb[:], in_=ones1c[:])
            fgw_sb = cp.tile([128, 2 * BS], bf16)
            nc.gpsimd.dma_start(out=fgw_sb[:], in_=fgw[:])
            fgb_sb = cp.tile([1, 2 * BS], bf16)
            nc.gpsimd.dma_start(out=fgb_sb[:], in_=fgb[:])
            wqc_sb = cp.tile([128, 2, NB, 128], bf16)
            nc.gpsimd.dma_start(out=wqc_sb[:], in_=wqc[:])
            wkc_sb = cp.tile([128, 2, NB, 128], bf16)
            nc.gpsimd.dma_start(out=wkc_sb[:], in_=wkc[:])
            wvc_sb = cp.tile([128, 2, NB, 128], bf16)
            nc.gpsimd.dma_start(out=wvc_sb[:], in_=wvc[:])
            cx_sb = [pp.tile([128, NHID], f32, tag=f"cx{bt}", name=f"cx{bt}")
                     for bt in range(2)]
            for bt in range(2):
                nc.gpsimd.dma_start(out=cx_sb[bt][:],
                                    in_=cx_bm[bt * 128:(bt + 1) * 128, :])
            hx_sb = [pp.tile([128, NHID], f32, tag=f"hx{bt}", name=f"hx{bt}")
                     for bt in range(2)]
            for bt in range(2):
                nc.gpsimd.dma_start(out=hx_sb[bt][:],
                                    in_=hx_bm[bt * 128:(bt + 1) * 128, :])

            xt8_sb = pp.tile([128, 16, 2, 128], fp8)
            xtb_sb = pp.tile([128, 16, 2, 128], bf16)
            # hnew_sb holds h_new per group, overwritten in place by
            # d0 = h_new - hx once hnb/hnewT snapshots are taken
            hnew_sb = [pp.tile([128, NHID], f32, tag=f"hn{bt}", name=f"hn{bt}")
                       for bt in range(2)]
            hnewT_sb = pp.tile([128, 16, BSH], bf16)
            mask_sb = [pp.tile([128, NB], f32, tag=f"mk{bt}", name=f"mk{bt}")
                       for bt in range(2)]
            sig_sb = [pp.tile([128, NB], f32, tag=f"sg{bt}", name=f"sg{bt}")
                      for bt in range(2)]
            qc_sb = pp.tile([128, NB, BSH], bf16)
            kc_sb = pp.tile([128, NB, BSH], bf16)
            vc_sb = pp.tile([128, NB, BSH], bf16)

            # ---- phase A (f32, mask-exact) -------------------------------
            with tc.tile_pool(name="pa", bufs=1) as pa, \
                 tc.tile_pool(name="pa2", bufs=2) as pa2, \
                 tc.tile_pool(name="paps", bufs=2, space="PSUM") as paps:
                # A inputs first on the scalar queue (arrive ~3us)
                inpT_sb = pa.tile([128, 8, BSH], f32)
                nc.scalar.dma_start(out=inpT_sb[:], in_=inpT[:])
                wk1_sb = pa.tile([128, 8, DKI], f32)
                nc.scalar.dma_start(out=wk1_sb[:], in_=wk1[:])
                wv1_sb = pa.tile([128, 8, BS], f32)
                nc.scalar.dma_start(out=wv1_sb[:], in_=wv1[:])
                wq_sb = pa.tile([128, 2, NB, DKI], f32)
                nc.scalar.dma_start(out=wq_sb[:], in_=wq[:])
                hxTf_sb = pa.tile([128, 16, BSH], f32)
                nc.scalar.dma_start(out=hxTf_sb[:], in_=hxT_f[:])
                biasbf_sb = cp.tile([1, 4096], bf16)
                nc.scalar.dma_start(out=biasbf_sb[:], in_=biasbf[:])

                for bt in range(2):
                    bsl = slice(bt * 128, (bt + 1) * 128)
                    k1_ps = paps.tile([128, DKI], f32, tag="k1")
                    for k in range(8):
                        nc.tensor.matmul(k1_ps[:], inpT_sb[:, k, bsl],
                                         wk1_sb[:, k, :],
                                         start=(k == 0), stop=(k == 7))
                    k1s = pa2.tile([128, DKI], f32, tag="k1s")
                    nc.vector.tensor_copy(k1s[:], k1_ps[:])

                    v1_ps = paps.tile([128, BS], f32, tag="v1")
                    for k in range(8):
                        nc.tensor.matmul(v1_ps[:], inpT_sb[:, k, bsl],
                                         wv1_sb[:, k, :],
                                         start=(k == 0), stop=(k == 7))
                    v1s = pa2.tile([128, BS], f32, tag="v1s")
                    nc.vector.tensor_copy(v1s[:], v1_ps[:])

                    q_ps = paps.tile([128, NB, DKI], f32, tag="q")
                    for n in range(NB):
                        for s in range(2):
                            nc.tensor.matmul(q_ps[:, n, :],
                                             hxTf_sb[:, 2 * n + s, bsl],
                                             wq_sb[:, s, n, :],
                                             start=(s == 0), stop=(s == 1))
                    prod = pa2.tile([128, NB, DKI], f32, tag="prod")
                    k1a = k1s[:]
                    k1bc = bass.AP(tensor=k1a.tensor, offset=k1a.offset,
                                   ap=[k1a.ap[0], [0, NB], k1a.ap[1]])
                    nc.vector.tensor_tensor(prod[:], q_ps[:], k1bc, OP.mult)
                    s1 = pa2.tile([128, NB], f32, tag="s1")
                    nc.vector.reduce_sum(s1[:], prod[:], axis=AX.X)
                    nc.scalar.activation(sig_sb[bt][:], s1[:], AF.Sigmoid,
                                         scale=0.125)

                    # top-4 mask
                    cnt = pa2.tile([128, NB], f32, tag="cnt")
                    tmp = pa2.tile([128, NB], f32, tag="tmp")
                    for n in range(NB):
                        nc.vector.tensor_single_scalar(tmp[:], s1[:],
                                                       s1[:, n:n + 1], OP.is_gt)
                        nc.vector.reduce_sum(cnt[:, n:n + 1], tmp[:], axis=AX.X)
                    nc.vector.tensor_single_scalar(mask_sb[bt][:], cnt[:], 4.0,
                                                   OP.is_lt)
                    nc.gpsimd.dma_start(out=mask_out[bsl, :], in_=mask_sb[bt][:])

                    # inp_flat -> feature-major (DMA transpose) + fp8 cast
                    ifl = pa2.tile([128, NB, BS], bf16, tag="ifl")
                    for n in range(NB):
                        nc.vector.tensor_single_scalar(ifl[:, n, :], v1s[:],
                                                       sig_sb[bt][:, n:n + 1],
                                                       OP.mult)
                    for ft in range(16):
                        eng = nc.sync if bt == 0 else nc.scalar
                        eng.dma_start(
                            out=xtb_sb[:, ft, bt, :],
                            in_=ifl[:, ft // 2, (ft % 2) * 128:(ft % 2) * 128 + 128],
                            transpose=True)
                        nc.vector.tensor_copy(xt8_sb[:, ft, bt, :],
                                              xtb_sb[:, ft, bt, :])

            # ---- phase B: LSTM groups, two per weight fetch --------------
            pair_order = list(range(8, 16)) + list(range(8))
            with tc.tile_pool(name="gps", bufs=1, space="PSUM") as gps, \
                 tc.tile_pool(name="pw", bufs=8) as pw, \
                 tc.tile_pool(name="pb2", bufs=2) as pb2:
                for gpair in range(4):
                    g_all = {}
                    for sub in range(2):
                        for bt in range(2):
                            g_all[sub, bt] = gps.tile(
                                [128, 4, BS], f32, tag=f"g{sub}{bt}",
                                name=f"g{sub}{bt}")
                    csl2 = slice(gpair * 1024, (gpair + 1) * 1024)
                    for sub in range(2):
                        gq = 2 * gpair + sub
                        csl = slice(gq * 512, (gq + 1) * 512)
                        for bt in range(2):
                            nc.tensor.matmul(g_all[sub, bt][:, 0:2, :],
                                             ones1_sb[:], bias8_sb[:, csl],
                                             start=True, stop=False)
                            nc.tensor.matmul(g_all[sub, bt][:, 2:4, :],
                                             ones1_sb[:], biasbf_sb[:, csl],
                                             start=True, stop=False)
                    for j in pair_order:
                        w8t = pw.tile([128, 2, 1024], fp8, tag="w8t")
                        nc.sync.dma_start(out=w8t[:],
                                          in_=w8d[:, 2 * j:2 * j + 2, csl2])
                        wbt = pw.tile([128, 2, 1024], bf16, tag="wbt")
                        nc.scalar.dma_start(out=wbt[:],
                                            in_=wbfd[:, 2 * j:2 * j + 2, csl2])
                        st = (j == pair_order[-1])
                        for sub in range(2):
                            off = sub * 512
                            for bt in range(2):
                                bsl = slice(bt * 128, (bt + 1) * 128)
                                if j >= 8:
                                    t = 2 * (j - 8)
                                    lhs8 = hxT8_sb[:, t:t + 2, bsl]
                                    lhsb = [hxTb_sb[:, t + kk, bsl]
                                            for kk in range(2)]
                                else:
                                    lhs8 = xt8_sb[:, 2 * j:2 * j + 2, bt, :]
                                    lhsb = [xtb_sb[:, 2 * j + kk, bt, :]
                                            for kk in range(2)]
                                nc.tensor.matmul(g_all[sub, bt][:, 0:2, :],
                                                 lhs8, w8t[:, :, off:off + 512],
                                                 start=False, stop=st,
                                                 perf_mode=DR)
                                for kk in range(2):
                                    nc.tensor.matmul(
                                        g_all[sub, bt][:, 2:4, :], lhsb[kk],
                                        wbt[:, kk, off:off + 512],
                                        start=False, stop=(st and kk == 1))
                    # ---- group tails ----------------------------------
                    for sub in range(2):
                        gq = 2 * gpair + sub
                        sl = slice(gq * BS, (gq + 1) * BS)
                        for bt in range(2):
                            sio = pb2.tile([128, 2, BS], f32, tag="sio",
                                           name="sio")
                            nc.scalar.activation(sio[:], g_all[sub, bt][:, 0:2, :],
                                                 AF.Sigmoid)
                            sigf = pb2.tile([128, BS], f32, tag="sigf",
                                            name="sigf")
                            nc.scalar.activation(sigf[:], g_all[sub, bt][:, 2, :],
                                                 AF.Sigmoid)
                            tang = pb2.tile([128, BS], f32, tag="tang",
                                            name="tang")
                            nc.scalar.activation(tang[:], g_all[sub, bt][:, 3, :],
                                                 AF.Tanh)
                            t1 = pb2.tile([128, BS], f32, tag="t1", name="t1")
                            nc.vector.tensor_tensor(t1[:], sigf[:],
                                                    cx_sb[bt][:, sl], OP.mult)
                            t2 = pb2.tile([128, BS], f32, tag="t2", name="t2")
                            nc.gpsimd.tensor_tensor(t2[:], sio[:, 0, :], tang[:],
                                                    OP.mult)
                            cnew = pb2.tile([128, BS], f32, tag="cnew",
                                            name="cnew")
                            nc.vector.tensor_tensor(cnew[:], t1[:], t2[:], OP.add)
                            t3 = pb2.tile([128, BS], f32, tag="t3", name="t3")
                            nc.scalar.activation(t3[:], cnew[:], AF.Tanh)
                            nc.vector.tensor_tensor(hnew_sb[bt][:, sl],
                                                    sio[:, 1, :], t3[:], OP.mult)
                            hnb = pb2.tile([128, BS], bf16, tag="hnb", name="hnb")
                            nc.gpsimd.tensor_copy(hnb[:], hnew_sb[bt][:, sl])
                            dc = pb2.tile([128, BS], f32, tag="dc", name="dc")
                            nc.gpsimd.tensor_tensor(dc[:], cnew[:],
                                                    cx_sb[bt][:, sl], OP.subtract)
                            co = pb2.tile([128, BS], f32, tag="co", name="co")
                            nc.vector.scalar_tensor_tensor(
                                co[:], dc[:], mask_sb[bt][:, gq:gq + 1],
                                cx_sb[bt][:, sl], OP.mult, OP.add)
                            nc.gpsimd.dma_start(
                                out=cx_out[bt * 128:(bt + 1) * 128, sl],
                                in_=co[:])
                            for s in range(2):
                                eng = nc.sync if bt == 0 else nc.scalar
                                eng.dma_start(
                                    out=hnewT_sb[:, 2 * gq + s,
                                                 bt * 128:(bt + 1) * 128],
                                    in_=hnb[:, s * 128:(s + 1) * 128],
                                    transpose=True)
                            # d0 = h_new - hx, in place (merge shortcut)
                            nc.gpsimd.tensor_tensor(hnew_sb[bt][:, sl],
                                                    hnew_sb[bt][:, sl],
                                                    hx_sb[bt][:, sl],
                                                    OP.subtract)

            # ============================ phase C ========================
            with tc.tile_pool(name="pc", bufs=1) as pc, \
                 tc.tile_pool(name="pctmp", bufs=3) as pctmp:
                at_sb = pc.tile([32, NB, BSH], bf16)
                coutb_sb = pc.tile([128, NB, BSH], bf16)
                with tc.tile_pool(name="psS", bufs=1, space="PSUM") as psS, \
                     tc.tile_pool(name="prjC", bufs=2, space="PSUM") as prj:
                    for wsb, dst in ((wkc_sb, kc_sb), (wvc_sb, vc_sb)):
                        for n in range(NB):
                            ps = prj.tile([128, BSH], f32, tag="proj")
                            for s in range(2):
                                nc.tensor.matmul(ps[:], wsb[:, s, n, :],
                                                 hnewT_sb[:, 2 * n + s, :],
                                                 start=(s == 0), stop=(s == 1))
                            nc.scalar.copy(dst[:, n, :], ps[:])
                    S = psS.tile([32, NB, BSH], f32, tag="S", name="S")
                    for q in range(NB):
                        ps = prj.tile([128, BSH], f32, tag="proj")
                        for s in range(2):
                            nc.tensor.matmul(ps[:], wqc_sb[:, s, q, :],
                                             hnewT_sb[:, 2 * q + s, :],
                                             start=(s == 0), stop=(s == 1))
                        nc.scalar.copy(qc_sb[:, q, :], ps[:])
                        pr = pctmp.tile([128, NB, BSH], bf16, tag="pr", name="pr")
                        qa = qc_sb[:, q, :]
                        qbc = bass.AP(tensor=qa.tensor, offset=qa.offset,
                                      ap=[qa.ap[0], [0, NB], qa.ap[-1]])
                        nc.vector.tensor_tensor(pr[:], qbc, kc_sb[:], OP.mult)
                        for kp in range(4):
                            nc.tensor.matmul(S[:, 2 * kp:2 * kp + 2, :],
                                             hq_sb[:, q, :],
                                             pr[:, 2 * kp:2 * kp + 2, :],
                                             start=(q == 0), stop=(q == 7))
                    ex = pc.tile([32, NB, BSH], bf16, tag="ex", name="ex")
                    nc.scalar.activation(ex[:], S[:], AF.Exp,
                                         scale=float(1.0 / np.sqrt(32.0)))
                    # denominator by bf16 tree adds (contiguous slices)
                    e1 = pctmp.tile([32, 4, BSH], bf16, tag="e1", name="e1")
                    nc.vector.tensor_tensor(e1[:], ex[:, 0:4, :], ex[:, 4:8, :],
                                            OP.add)
                    e2 = pctmp.tile([32, 2, BSH], bf16, tag="e2", name="e2")
                    nc.vector.tensor_tensor(e2[:], e1[:, 0:2, :], e1[:, 2:4, :],
                                            OP.add)
                    denom = pctmp.tile([32, BSH], f32, tag="denom", name="denom")
                    nc.vector.tensor_tensor(denom[:], e2[:, 0, :], e2[:, 1, :],
                                            OP.add)
                    recip = pctmp.tile([32, BSH], f32, tag="recip", name="recip")
                    nc.vector.reciprocal(recip[:], denom[:])
                    ra = recip[:]
                    rbc = bass.AP(tensor=ra.tensor, offset=ra.offset,
                                  ap=[ra.ap[0], [0, NB], ra.ap[-1]])
                    nc.vector.tensor_tensor(at_sb[:], ex[:], rbc, OP.mult)

                with tc.tile_pool(name="psU", bufs=1, space="PSUM") as psU, \
                     tc.tile_pool(name="psOG", bufs=2, space="PSUM") as psOG:
                    for q in range(NB):
                        U = psU.tile([128, NB, BSH], f32, tag="U", name="U")
                        for kp in range(4):
                            nc.tensor.matmul(U[:, 2 * kp:2 * kp + 2, :],
                                             e32_sb[:, q, :],
                                             at_sb[:, 2 * kp:2 * kp + 2, :],
                                             start=True, stop=True)
                        prods = pctmp.tile([128, NB, BSH], bf16, tag="prods")
                        nc.vector.tensor_tensor(prods[:], U[:], vc_sb[:], OP.mult)
                        tr1 = pctmp.tile([128, 4, BSH], bf16, tag="tr1",
                                         name="tr1")
                        nc.vector.tensor_tensor(tr1[:], prods[:, 0:4, :],
                                                prods[:, 4:8, :], OP.add)
                        tr2 = pctmp.tile([128, 2, BSH], bf16, tag="tr2",
                                         name="tr2")
                        nc.vector.tensor_tensor(tr2[:], tr1[:, 0:2, :],
                                                tr1[:, 2:4, :], OP.add)
                        nc.vector.tensor_tensor(coutb_sb[:, q, :], tr2[:, 0, :],
                                                tr2[:, 1, :], OP.add)
                        # gated residual + masked merge for this block
                        for bt in range(2):
                            csl = coutb_sb[:, q, bt * 128:(bt + 1) * 128]
                            og = psOG.tile([128, 2 * BS], f32, tag="og",
                                           name="og")
                            nc.tensor.matmul(og[:], csl, fgw_sb[:],
                                             start=True, stop=False)
                            nc.tensor.matmul(og[:], ones1_sb[:], fgb_sb[:],
                                             start=False, stop=True)
                            tano = pctmp.tile([128, BS], f32, tag="tano",
                                              name="tano")
                            nc.scalar.activation(tano[:], og[:, 0:BS], AF.Tanh)
                            sg = pctmp.tile([128, BS], f32, tag="sgx", name="sgx")
                            nc.scalar.activation(sg[:], og[:, BS:2 * BS],
                                                 AF.Sigmoid)
                            hatt = pctmp.tile([128, BS], f32, tag="hatt",
                                              name="hatt")
                            nc.vector.tensor_tensor(hatt[:], sg[:], tano[:],
                                                    OP.mult)
                            qsl = slice(q * BS, (q + 1) * BS)
                            # dh = d0 + hatt ; ho = mask*dh + hx
                            dh = pctmp.tile([128, BS], f32, tag="dhq", name="dhq")
                            nc.gpsimd.tensor_tensor(dh[:], hnew_sb[bt][:, qsl],
                                                    hatt[:], OP.add)
                            ho = pctmp.tile([128, BS], f32, tag="hoq", name="hoq",
                                            bufs=4)
                            nc.vector.scalar_tensor_tensor(ho[:], dh[:],
                                                           mask_sb[bt][:, q:q + 1],
                                                           hx_sb[bt][:, qsl],
                                                           OP.mult, OP.add)
                            nc.gpsimd.dma_start(
                                out=hx_out[bt * 128:(bt + 1) * 128, qsl],
                                in_=ho[:])

    _install_bir_fix(nc)
    return nc


# ---------------------------------------------------------------------------
# Host wrapper
# ---------------------------------------------------------------------------

def kernel(inp, hx, cx, wq_inp, wk_inp, wv_inp, w_ih, w_hh, b_ih, b_hh,
           wq_c, wk_c, wv_c, fc_w, fc_b, gate_w, gate_b, step=None):
    global last_exec_time_ns, last_results

    inp = np.asarray(inp, np.float32)
    hx = np.asarray(hx, np.float32)
    cx = np.asarray(cx, np.float32)
    wq_inp = np.asarray(wq_inp, np.float32)
    wk_inp = np.asarray(wk_inp, np.float32)
    wv_inp = np.asarray(wv_inp, np.float32)
    w_ih = np.asarray(w_ih, np.float32)
    w_hh = np.asarray(w_hh, np.float32)
    b_ih = np.asarray(b_ih, np.float32)
    b_hh = np.asarray(b_hh, np.float32)
    wq_c = np.asarray(wq_c, np.float32)
    wk_c = np.asarray(wk_c, np.float32)
    wv_c = np.asarray(wv_c, np.float32)
    fc_w = np.asarray(fc_w, np.float32)
    fc_b = np.asarray(fc_b, np.float32)
    gate_w = np.asarray(gate_w, np.float32)
    gate_b = np.asarray(gate_b, np.float32)

    if "nc" not in _CACHE:
        _CACHE["nc"] = _build()
    nc = _CACHE["nc"]

    # column permutations: per 256-wide hidden group g the fp8 panel holds
    # [i|o], the bf16 panel [f|g]  (torch gate order i,f,g,o)
    wcat = np.concatenate([w_ih.T, w_hh.T], axis=0)     # (4096, 8192)
    bias = (b_ih + b_hh)
    perm8 = np.concatenate([np.concatenate([
        np.arange(0 * NHID + g * BS, 0 * NHID + (g + 1) * BS),
        np.arange(3 * NHID + g * BS, 3 * NHID + (g + 1) * BS)])
        for g in range(8)])
    permbf = np.concatenate([np.concatenate([
        np.arange(1 * NHID + g * BS, 1 * NHID + (g + 1) * BS),
        np.arange(2 * NHID + g * BS, 2 * NHID + (g + 1) * BS)])
        for g in range(8)])
    w8_np = wcat[:, perm8].astype(E4)                   # (4096, 4096)
    wbf_np = wcat[:, permbf].astype(BF16)
    w8d = np.ascontiguousarray(w8_np.reshape(32, 128, 4096).transpose(1, 0, 2))
    wbfd = np.ascontiguousarray(wbf_np.reshape(32, 128, 4096).transpose(1, 0, 2))

    shared = {
        "wq": np.ascontiguousarray(
            wq_inp.reshape(NB, 2, 128, DKI).transpose(2, 1, 0, 3)),
        "wk1": np.ascontiguousarray(
            wk_inp[1].reshape(8, 128, DKI).transpose(1, 0, 2)),
        "wv1": np.ascontiguousarray(
            wv_inp[1].reshape(8, 128, BS).transpose(1, 0, 2)),
        "w8d": w8d,
        "wbfd": wbfd,
        "bias8": bias[perm8].astype(BF16).reshape(1, 4096),
        "biasbf": bias[permbf].astype(BF16).reshape(1, 4096),
        "wqc": np.ascontiguousarray(
            wq_c.astype(BF16).reshape(NB, 2, 128, 128).transpose(2, 1, 0, 3)),
        "wkc": np.ascontiguousarray(
            wk_c.astype(BF16).reshape(NB, 2, 128, 128).transpose(2, 1, 0, 3)),
        "wvc": np.ascontiguousarray(
            wv_c.astype(BF16).reshape(NB, 2, 128, 128).transpose(2, 1, 0, 3)),
        "fgw": np.ascontiguousarray(
            np.concatenate([fc_w, gate_w], axis=1)).astype(BF16),
        "fgb": np.concatenate([fc_b, gate_b]).astype(BF16).reshape(1, 2 * BS),
    }

    in_maps = []
    for c in range(NCORES):
        rs = slice(c * BSH, (c + 1) * BSH)
        inpT_c = inp[rs].T.reshape(8, 128, BSH).transpose(1, 0, 2)
        hxT = hx[rs].T.reshape(16, 128, BSH).transpose(1, 0, 2)
        m = {
            "inpT": np.ascontiguousarray(inpT_c),
            "hxT_f": np.ascontiguousarray(hxT),
            "hxT_b": np.ascontiguousarray(hxT.astype(BF16)),
            "hxT_8": np.ascontiguousarray(hxT.astype(E4)),
            "hx_bm": np.ascontiguousarray(hx[rs]),
            "cx_bm": np.ascontiguousarray(cx[rs]),
        }
        m.update(shared)
        in_maps.append(m)

    from concourse.bass_utils import run_bass_kernel_spmd
    trace = bool(int(os.environ.get("BASS_KTRACE", "0")))
    res = run_bass_kernel_spmd(nc, in_maps, list(range(NCORES)), trace=trace)
    last_exec_time_ns = res.exec_time_ns
    last_results = res

    hx_full = np.empty((B, NHID), np.float32)
    cx_full = np.empty((B, NHID), np.float32)
    mask_full = np.empty((B, NHID), np.float32)
    for c in range(NCORES):
        rs = slice(c * BSH, (c + 1) * BSH)
        hx_full[rs] = res.results[c]["hx_out"]
        cx_full[rs] = res.results[c]["cx_out"]
        mask_full[rs] = np.repeat(res.results[c]["mask_out"], BS, axis=1)
    return hx_full, cx_full, mask_full


# revision 16
# speedup vs baseline: 1.1351x; 1.1254x over previous
"""Trainium2 Bass kernel for nn_BlocksCore (RIMs BlocksCore step).

Data-parallel over batch B=2048 across 8 NeuronCores (256 rows each),
parameters replicated. Per-core plan (v3):

  A. input attention in f32 (mask-exact); inp_flat transposed to
     feature-major via DMA-transpose (bf16) + fp8 cast copies.
  B. LSTM gates: i,o via fp8e4 DoubleRow matmuls, f,g via bf16, processed
     per 256-wide hidden group (== one attention block); weight panels
     fetched two groups at a time ([128,2,1024] tiles) on the sync (fp8)
     and scalar (bf16) HWDGE queues; group tails (activations, c/h update,
     cx merge, h_new^T DMA-transpose) pipeline under the next group's
     matmuls. d0 = h_new - hx precomputed for the final merge.
  C. communication attention: q/k/v projections, one 32-row score tile for
     all (head, q-block) pairs, single softmax, PE-expanded apply with
     bf16 tree reductions, gated residual + masked merge per block.

Outputs: hx_out/cx_out [256,2048] f32, mask_out [256,8] (host expands).
"""

import json
import os

import numpy as np
import ml_dtypes

BF16 = ml_dtypes.bfloat16
E4 = ml_dtypes.float8_e4m3

B = 2048
NCORES = 8
BSH = B // NCORES          # 256 batch rows per core
NINP = 1024
NHID = 2048
NB = 8                     # blocks
BS = 256                   # block size (NHID / NB)
DKI = 64                   # input-attention d_k

_CACHE = {}
last_exec_time_ns = None
last_results = None

# ---------------------------------------------------------------------------
# BIR post-fix: this toolchain's core_v3 codegen supports only one sync-wait
# per CTRL-class instruction; hoist extras onto single-wait EventSemaphores.
# ---------------------------------------------------------------------------


def _fix_bir_json(bir_bytes: bytes) -> bytes:
    bir = json.loads(bir_bytes)
    for fn in bir.get("functions", []):
        for blk in fn.get("blocks", []):
            out = []
            for ins in blk.get("instructions", []):
                si = ins.get("sync_info") or {}
                waits = si.get("on_wait") or []
                if len(waits) > 1:
                    for j, w in enumerate(waits[:-1]):
                        out.append({
                            "name": f"{ins['name']}-w{j}",
                            "engine": ins["engine"],
                            "opcode": "EventSemaphore",
                            "ins": [],
                            "outs": [],
                            "sync_info": {"on_update": [], "on_wait": [w]},
                        })
                    si = dict(si)
                    si["on_wait"] = [waits[-1]]
                    ins = dict(ins)
                    ins["sync_info"] = si
                out.append(ins)
            blk["instructions"] = out
    return json.dumps(bir).encode()


def _install_bir_fix(nc):
    orig = nc.to_json_bytes

    def patched(*a, **k):
        return _fix_bir_json(orig(*a, **k))

    nc.to_json_bytes = patched


# ---------------------------------------------------------------------------
# Device kernel
# ---------------------------------------------------------------------------

def _build():
    import concourse.bass as bass
    import concourse.tile as tile
    from concourse import mybir

    f32 = mybir.dt.float32
    bf16 = mybir.dt.bfloat16
    fp8 = mybir.dt.float8e4
    OP = mybir.AluOpType
    AF = mybir.ActivationFunctionType
    AX = mybir.AxisListType
    DR = mybir.MatmulPerfMode.DoubleRow

    nc = bass.Bass()

    # ---- I/O ------------------------------------------------------------
    inpT = nc.declare_dram_parameter("inpT", [128, 8, BSH], f32, isOutput=False)
    hxT_f = nc.declare_dram_parameter("hxT_f", [128, 16, BSH], f32, isOutput=False)
    hxT_b = nc.declare_dram_parameter("hxT_b", [128, 16, BSH], bf16, isOutput=False)
    hxT_8 = nc.declare_dram_parameter("hxT_8", [128, 16, BSH], fp8, isOutput=False)
    hx_bm = nc.declare_dram_parameter("hx_bm", [BSH, NHID], f32, isOutput=False)
    cx_bm = nc.declare_dram_parameter("cx_bm", [BSH, NHID], f32, isOutput=False)
    wq = nc.declare_dram_parameter("wq", [128, 2, NB, DKI], f32, isOutput=False)
    wk1 = nc.declare_dram_parameter("wk1", [128, 8, DKI], f32, isOutput=False)
    wv1 = nc.declare_dram_parameter("wv1", [128, 8, BS], f32, isOutput=False)
    # LSTM weights: [128, 32 ktiles, 8 groups * 512] — per group g the fp8
    # panel holds [i|o] columns for hidden chunk g, the bf16 panel [f|g].
    w8d = nc.declare_dram_parameter("w8d", [128, 32, 4096], fp8, isOutput=False)
    wbfd = nc.declare_dram_parameter("wbfd", [128, 32, 4096], bf16, isOutput=False)
    bias8 = nc.declare_dram_parameter("bias8", [1, 4096], bf16, isOutput=False)
    biasbf = nc.declare_dram_parameter("biasbf", [1, 4096], bf16, isOutput=False)
    wqc = nc.declare_dram_parameter("wqc", [128, 2, NB, 128], bf16, isOutput=False)
    wkc = nc.declare_dram_parameter("wkc", [128, 2, NB, 128], bf16, isOutput=False)
    wvc = nc.declare_dram_parameter("wvc", [128, 2, NB, 128], bf16, isOutput=False)
    fgw = nc.declare_dram_parameter("fgw", [128, 2 * BS], bf16, isOutput=False)
    fgb = nc.declare_dram_parameter("fgb", [1, 2 * BS], bf16, isOutput=False)
    hx_out = nc.declare_dram_parameter("hx_out", [BSH, NHID], f32, isOutput=True)
    cx_out = nc.declare_dram_parameter("cx_out", [BSH, NHID], f32, isOutput=True)
    mask_out = nc.declare_dram_parameter("mask_out", [BSH, NB], f32, isOutput=True)

    # ---- inline constants ----------------------------------------------
    hq_np = np.zeros((128, NB, 32), dtype=BF16)
    for d in range(128):
        for q in range(NB):
            hq_np[d, q, (d // 32) * 8 + q] = 1
    e32_np = np.zeros((32, NB, 128), dtype=BF16)
    for m in range(128):
        for q in range(NB):
            e32_np[(m // 32) * 8 + q, q, m] = 1
    # partition broadcaster: sel8[n', n, p] = (n' == n); a K=8 matmul with
    # lhsT=sel8[:, n, :] replicates row n of the rhs across 128 partitions
    sel8_np = np.zeros((8, NB, 128), dtype=BF16)
    for n in range(NB):
        sel8_np[n, n, :] = 1
    hqc = nc.inline_tensor(hq_np, "hqc")
    e32b = nc.inline_tensor(e32_np, "e32b")
    ones1c = nc.inline_tensor(np.ones((1, 128), dtype=BF16), "ones1c")
    sel8c = nc.inline_tensor(sel8_np, "sel8c")
    identc = nc.inline_tensor(np.eye(128, dtype=BF16), "identc")

    with tile.TileContext(nc) as tc:
        with tc.tile_pool(name="cp", bufs=1) as cp, \
             tc.tile_pool(name="pp", bufs=1) as pp:
            # fast-path inputs on sync (needed within ~5us)
            bias8_sb = cp.tile([1, 4096], bf16)
            nc.sync.dma_start(out=bias8_sb[:], in_=bias8[:])
            hxT8_sb = pp.tile([128, 16, BSH], fp8)
            nc.sync.dma_start(out=hxT8_sb[:], in_=hxT_8[:])
            hxTb_sb = pp.tile([128, 16, BSH], bf16)
            nc.sync.dma_start(out=hxTb_sb[:], in_=hxT_b[:])

            # constants and late inputs on gpsimd
            hq_sb = cp.tile([128, NB, 32], bf16)
            nc.gpsimd.dma_start(out=hq_sb[:], in_=hqc[:])
            e32_sb = cp.tile([32, NB, 128], bf16)
            nc.gpsimd.dma_start(out=e32_sb[:], in_=e32b[:])
            ones1_sb = cp.tile([1, 128], bf16)
            nc.gpsimd.dma_start(out=ones1_sb[:], in_=ones1c[:])
            fgw_sb = cp.tile([128, 2 * BS], bf16)
            nc.gpsimd.dma_start(out=fgw_sb[:], in_=fgw[:])
            fgb_sb = cp.tile([1, 2 * BS], bf16)
            nc.gpsimd.dma_start(out=fgb_sb[:], in_=fgb[:])
            wqc_sb = cp.tile([128, 2, NB, 128], bf16)
            nc.gpsimd.dma_start(out=wqc_sb[:], in_=wqc[:])
            wkc_sb = cp.tile([128, 2, NB, 128], bf16)
            nc.gpsimd.dma_start(out=wkc_sb[:], in_=wkc[:])
            wvc_sb = cp.tile([128, 2, NB, 128], bf16)
            nc.gpsimd.dma_start(out=wvc_sb[:], in_=wvc[:])
            cx_sb = [pp.tile([128, NHID], f32, tag=f"cx{bt}", name=f"cx{bt}")
                     for bt in range(2)]
            for bt in range(2):
                nc.gpsimd.dma_start(out=cx_sb[bt][:],
                                    in_=cx_bm[bt * 128:(bt + 1) * 128, :])
            hx_sb = [pp.tile([128, NHID], f32, tag=f"hx{bt}", name=f"hx{bt}")
                     for bt in range(2)]
            for bt in range(2):
                nc.gpsimd.dma_start(out=hx_sb[bt][:],
                                    in_=hx_bm[bt * 128:(bt + 1) * 128, :])

            sel8_sb = cp.tile([8, NB, 128], bf16)
            nc.gpsimd.dma_start(out=sel8_sb[:], in_=sel8c[:])
            ident_sb = cp.tile([128, 128], bf16)
            nc.gpsimd.dma_start(out=ident_sb[:], in_=identc[:])
            xt8_sb = pp.tile([128, 16, BSH], fp8)
            xtb_sb = pp.tile([128, 16, BSH], bf16)
            # hnew_sb holds h_new per group, overwritten in place by
            # d0 = h_new - hx once hnb/hnewT snapshots are taken
            hnew_sb = [pp.tile([128, NHID], f32, tag=f"hn{bt}", name=f"hn{bt}")
                       for bt in range(2)]
            hnewT_sb = pp.tile([128, 16, BSH], bf16)
            mask_sb = [pp.tile([128, NB], f32, tag=f"mk{bt}", name=f"mk{bt}")
                       for bt in range(2)]
            sig_sb = [pp.tile([128, NB], bf16, tag=f"sg{bt}", name=f"sg{bt}")
                      for bt in range(2)]
            qc_sb = pp.tile([128, NB, BSH], bf16)
            kc_sb = pp.tile([128, NB, BSH], bf16)
            vc_sb = pp.tile([128, NB, BSH], bf16)

            # ---- phase A (f32, mask-exact) -------------------------------
            with tc.tile_pool(name="pa", bufs=1) as pa, \
                 tc.tile_pool(name="pa2", bufs=2) as pa2, \
                 tc.tile_pool(name="paps", bufs=1, space="PSUM") as paps:
                # A inputs first on the scalar queue (arrive ~3us)
                inpT_sb = pa.tile([128, 8, BSH], f32)
                nc.scalar.dma_start(out=inpT_sb[:], in_=inpT[:])
                wk1_sb = pa.tile([128, 8, DKI], f32)
                nc.scalar.dma_start(out=wk1_sb[:], in_=wk1[:])
                wv1_sb = pa.tile([128, 8, BS], f32)
                nc.scalar.dma_start(out=wv1_sb[:], in_=wv1[:])
                wq_sb = pa.tile([128, 2, NB, DKI], f32)
                nc.scalar.dma_start(out=wq_sb[:], in_=wq[:])
                hxTf_sb = pa.tile([128, 16, BSH], f32)
                nc.scalar.dma_start(out=hxTf_sb[:], in_=hxT_f[:])
                biasbf_sb = cp.tile([1, 4096], bf16)
                nc.scalar.dma_start(out=biasbf_sb[:], in_=biasbf[:])

                # v1^T = wv1^T @ inp^T directly in feature-major (both halves)
                v1T_sb = pa.tile([128, 2, BSH], f32)
                for s in range(2):
                    v1T_ps = paps.tile([128, BSH], f32, tag="v1T")
                    for k in range(8):
                        nc.tensor.matmul(v1T_ps[:],
                                         wv1_sb[:, k, s * 128:(s + 1) * 128],
                                         inpT_sb[:, k, :],
                                         start=(k == 0), stop=(k == 7))
                    nc.vector.tensor_copy(v1T_sb[:, s, :], v1T_ps[:])

                sigT_sb = pa.tile([8, BSH], bf16)
                for bt in range(2):
                    bsl = slice(bt * 128, (bt + 1) * 128)
                    k1_ps = paps.tile([128, DKI], f32, tag="k1")
                    for k in range(8):
                        nc.tensor.matmul(k1_ps[:], inpT_sb[:, k, bsl],
                                         wk1_sb[:, k, :],
                                         start=(k == 0), stop=(k == 7))
                    k1s = pa2.tile([128, DKI], f32, tag="k1s")
                    nc.vector.tensor_copy(k1s[:], k1_ps[:])

                    q_ps = paps.tile([128, NB, DKI], f32, tag="q")
                    for n in range(NB):
                        for s in range(2):
                            nc.tensor.matmul(q_ps[:, n, :],
                                             hxTf_sb[:, 2 * n + s, bsl],
                                             wq_sb[:, s, n, :],
                                             start=(s == 0), stop=(s == 1))
                    prod = pa2.tile([128, NB, DKI], f32, tag="prod")
                    k1a = k1s[:]
                    k1bc = bass.AP(tensor=k1a.tensor, offset=k1a.offset,
                                   ap=[k1a.ap[0], [0, NB], k1a.ap[1]])
                    nc.vector.tensor_tensor(prod[:], q_ps[:], k1bc, OP.mult)
                    s1 = pa2.tile([128, NB], f32, tag="s1")
                    nc.vector.reduce_sum(s1[:], prod[:], axis=AX.X)
                    nc.scalar.activation(sig_sb[bt][:], s1[:], AF.Sigmoid,
                                         scale=0.125)

                    # top-4 mask (rank counts fused via accum_out)
                    cnt = pa2.tile([128, NB], f32, tag="cnt")
                    tmp = pa2.tile([128, NB], f32, tag="tmp")
                    for n in range(NB):
                        nc.vector.tensor_scalar(tmp[:], s1[:], s1[:, n:n + 1],
                                                0.0, OP.is_gt, OP.add,
                                                accum_out=cnt[:, n:n + 1])
                    nc.vector.tensor_single_scalar(mask_sb[bt][:], cnt[:], 4.0,
                                                   OP.is_lt)
                    nc.gpsimd.dma_start(out=mask_out[bsl, :], in_=mask_sb[bt][:])
                    # sig^T half for the partition broadcast below
                    sgt = paps.tile([8, 128], bf16, tag="sgt")
                    nc.tensor.transpose(sgt[:], sig_sb[bt][:], ident_sb[:])
                    nc.vector.tensor_copy(sigT_sb[:, bsl], sgt[:])

                # inp_flat^T = v1^T * broadcast(sig^T) per block, cast to
                # bf16 (f,g path) and fp8 (DoubleRow path)
                with tc.tile_pool(name="pasg", bufs=2, space="PSUM") as pasg:
                    for n in range(NB):
                        sgb = pasg.tile([128, BSH], f32, tag="sgb")
                        nc.tensor.matmul(sgb[:], sel8_sb[:, n, :], sigT_sb[:],
                                         start=True, stop=True)
                        for s in range(2):
                            nc.vector.tensor_tensor(xtb_sb[:, 2 * n + s, :],
                                                    v1T_sb[:, s, :], sgb[:],
                                                    OP.mult)
                            nc.vector.tensor_tensor(xt8_sb[:, 2 * n + s, :],
                                                    v1T_sb[:, s, :], sgb[:],
                                                    OP.mult)

            # ---- phase B: LSTM groups, two per weight fetch --------------
            pair_order = list(range(8, 16)) + list(range(8))
            with tc.tile_pool(name="gps", bufs=1, space="PSUM") as gps, \
                 tc.tile_pool(name="pw", bufs=8) as pw, \
                 tc.tile_pool(name="pb2", bufs=2) as pb2:
                for gpair in range(4):
                    g_all = {}
                    for sub in range(2):
                        for bt in range(2):
                            g_all[sub, bt] = gps.tile(
                                [128, 4, BS], f32, tag=f"g{sub}{bt}",
                                name=f"g{sub}{bt}")
                    csl2 = slice(gpair * 1024, (gpair + 1) * 1024)
                    for sub in range(2):
                        gq = 2 * gpair + sub
                        csl = slice(gq * 512, (gq + 1) * 512)
                        for bt in range(2):
                            nc.tensor.matmul(g_all[sub, bt][:, 0:2, :],
                                             ones1_sb[:], bias8_sb[:, csl],
                                             start=True, stop=False)
                            nc.tensor.matmul(g_all[sub, bt][:, 2:4, :],
                                             ones1_sb[:], biasbf_sb[:, csl],
                                             start=True, stop=False)
                    for j in pair_order:
                        w8t = pw.tile([128, 2, 1024], fp8, tag="w8t")
                        nc.sync.dma_start(out=w8t[:],
                                          in_=w8d[:, 2 * j:2 * j + 2, csl2])
                        wbt = pw.tile([128, 2, 1024], bf16, tag="wbt")
                        nc.sync.dma_start(out=wbt[:],
                                          in_=wbfd[:, 2 * j:2 * j + 2, csl2])
                        st = (j == pair_order[-1])
                        for sub in range(2):
                            off = sub * 512
                            for bt in range(2):
                                bsl = slice(bt * 128, (bt + 1) * 128)
                                if j >= 8:
                                    t = 2 * (j - 8)
                                    lhs8 = hxT8_sb[:, t:t + 2, bsl]
                                    lhsb = [hxTb_sb[:, t + kk, bsl]
                                            for kk in range(2)]
                                else:
                                    lhs8 = xt8_sb[:, 2 * j:2 * j + 2, bsl]
                                    lhsb = [xtb_sb[:, 2 * j + kk, bsl]
                                            for kk in range(2)]
                                nc.tensor.matmul(g_all[sub, bt][:, 0:2, :],
                                                 lhs8, w8t[:, :, off:off + 512],
                                                 start=False, stop=st,
                                                 perf_mode=DR)
                                for kk in range(2):
                                    nc.tensor.matmul(
                                        g_all[sub, bt][:, 2:4, :], lhsb[kk],
                                        wbt[:, kk, off:off + 512],
                                        start=False, stop=(st and kk == 1))
                    # ---- group tails ----------------------------------
                    for sub in range(2):
                        gq = 2 * gpair + sub
                        sl = slice(gq * BS, (gq + 1) * BS)
                        for bt in range(2):
                            sio = pb2.tile([128, 2, BS], f32, tag="sio",
                                           name="sio")
                            nc.scalar.activation(sio[:], g_all[sub, bt][:, 0:2, :],
                                                 AF.Sigmoid)
                            sigf = pb2.tile([128, BS], f32, tag="sigf",
                                            name="sigf")
                            nc.scalar.activation(sigf[:], g_all[sub, bt][:, 2, :],
                                                 AF.Sigmoid)
                            tang = pb2.tile([128, BS], f32, tag="tang",
                                            name="tang")
                            nc.scalar.activation(tang[:], g_all[sub, bt][:, 3, :],
                                                 AF.Tanh)
                            t1 = pb2.tile([128, BS], f32, tag="t1", name="t1")
                            nc.vector.tensor_tensor(t1[:], sigf[:],
                                                    cx_sb[bt][:, sl], OP.mult)
                            t2 = pb2.tile([128, BS], f32, tag="t2", name="t2")
                            nc.gpsimd.tensor_tensor(t2[:], sio[:, 0, :], tang[:],
                                                    OP.mult)
                            cnew = pb2.tile([128, BS], f32, tag="cnew",
                                            name="cnew")
                            nc.vector.tensor_tensor(cnew[:], t1[:], t2[:], OP.add)
                            t3 = pb2.tile([128, BS], f32, tag="t3", name="t3")
                            nc.scalar.activation(t3[:], cnew[:], AF.Tanh)
                            nc.vector.tensor_tensor(hnew_sb[bt][:, sl],
                                                    sio[:, 1, :], t3[:], OP.mult)
                            hnb = pb2.tile([128, BS], bf16, tag="hnb", name="hnb")
                            nc.scalar.copy(hnb[:], hnew_sb[bt][:, sl])
                            dc = pb2.tile([128, BS], f32, tag="dc", name="dc")
                            nc.gpsimd.tensor_tensor(dc[:], cnew[:],
                                                    cx_sb[bt][:, sl], OP.subtract)
                            co = pb2.tile([128, BS], f32, tag="co", name="co")
                            nc.vector.scalar_tensor_tensor(
                                co[:], dc[:], mask_sb[bt][:, gq:gq + 1],
                                cx_sb[bt][:, sl], OP.mult, OP.add)
                            nc.gpsimd.dma_start(
                                out=cx_out[bt * 128:(bt + 1) * 128, sl],
                                in_=co[:])
                            for s in range(2):
                                nc.scalar.dma_start(
                                    out=hnewT_sb[:, 2 * gq + s,
                                                 bt * 128:(bt + 1) * 128],
                                    in_=hnb[:, s * 128:(s + 1) * 128],
                                    transpose=True)
                            # d0 = h_new - hx, in place (merge shortcut)
                            nc.gpsimd.tensor_tensor(hnew_sb[bt][:, sl],
                                                    hnew_sb[bt][:, sl],
                                                    hx_sb[bt][:, sl],
                                                    OP.subtract)

            # ============================ phase C ========================
            with tc.tile_pool(name="pc", bufs=1) as pc, \
                 tc.tile_pool(name="pctmp", bufs=3) as pctmp:
                at_sb = pc.tile([32, NB, BSH], bf16)
                coutb_sb = pc.tile([128, NB, BSH], bf16)
                with tc.tile_pool(name="psS", bufs=1, space="PSUM") as psS, \
                     tc.tile_pool(name="prjC", bufs=2, space="PSUM") as prj:
                    for wsb, dst in ((wkc_sb, kc_sb), (wvc_sb, vc_sb)):
                        for n in range(NB):
                            ps = prj.tile([128, BSH], f32, tag="proj")
                            for s in range(2):
                                nc.tensor.matmul(ps[:], wsb[:, s, n, :],
                                                 hnewT_sb[:, 2 * n + s, :],
                                                 start=(s == 0), stop=(s == 1))
                            nc.scalar.copy(dst[:, n, :], ps[:])
                    S = psS.tile([32, NB, BSH], f32, tag="S", name="S")
                    for q in range(NB):
                        ps = prj.tile([128, BSH], f32, tag="proj")
                        for s in range(2):
                            nc.tensor.matmul(ps[:], wqc_sb[:, s, q, :],
                                             hnewT_sb[:, 2 * q + s, :],
                                             start=(s == 0), stop=(s == 1))
                        nc.scalar.copy(qc_sb[:, q, :], ps[:])
                        pr = pctmp.tile([128, NB, BSH], bf16, tag="pr", name="pr")
                        qa = qc_sb[:, q, :]
                        qbc = bass.AP(tensor=qa.tensor, offset=qa.offset,
                                      ap=[qa.ap[0], [0, NB], qa.ap[-1]])
                        nc.vector.tensor_tensor(pr[:], qbc, kc_sb[:], OP.mult)
                        for kp in range(4):
                            nc.tensor.matmul(S[:, 2 * kp:2 * kp + 2, :],
                                             hq_sb[:, q, :],
                                             pr[:, 2 * kp:2 * kp + 2, :],
                                             start=(q == 0), stop=(q == 7))
                    ex = pc.tile([32, NB, BSH], bf16, tag="ex", name="ex")
                    nc.scalar.activation(ex[:], S[:], AF.Exp,
                                         scale=float(1.0 / np.sqrt(32.0)))
                    # denominator by bf16 tree adds (contiguous slices)
                    e1 = pctmp.tile([32, 4, BSH], bf16, tag="e1", name="e1")
                    nc.vector.tensor_tensor(e1[:], ex[:, 0:4, :], ex[:, 4:8, :],
                                            OP.add)
                    e2 = pctmp.tile([32, 2, BSH], bf16, tag="e2", name="e2")
                    nc.vector.tensor_tensor(e2[:], e1[:, 0:2, :], e1[:, 2:4, :],
                                            OP.add)
                    denom = pctmp.tile([32, BSH], f32, tag="denom", name="denom")
                    nc.vector.tensor_tensor(denom[:], e2[:, 0, :], e2[:, 1, :],
                                            OP.add)
                    recip = pctmp.tile([32, BSH], f32, tag="recip", name="recip")
                    nc.vector.reciprocal(recip[:], denom[:])
                    ra = recip[:]
                    rbc = bass.AP(tensor=ra.tensor, offset=ra.offset,
                                  ap=[ra.ap[0], [0, NB], ra.ap[-1]])
                    nc.vector.tensor_tensor(at_sb[:], ex[:], rbc, OP.mult)

                with tc.tile_pool(name="psU", bufs=1, space="PSUM") as psU, \
                     tc.tile_pool(name="psOG", bufs=2, space="PSUM") as psOG:
                    for q in range(NB):
                        U = psU.tile([128, NB, BSH], f32, tag="U", name="U")
                        for kp in range(4):
                            nc.tensor.matmul(U[:, 2 * kp:2 * kp + 2, :],
                                             e32_sb[:, q, :],
                                             at_sb[:, 2 * kp:2 * kp + 2, :],
                                             start=True, stop=True)
                        prods = pctmp.tile([128, NB, BSH], bf16, tag="prods")
                        nc.vector.tensor_tensor(prods[:], U[:], vc_sb[:], OP.mult)
                        tr1 = pctmp.tile([128, 4, BSH], bf16, tag="tr1",
                                         name="tr1")
                        nc.vector.tensor_tensor(tr1[:], prods[:, 0:4, :],
                                                prods[:, 4:8, :], OP.add)
                        tr2 = pctmp.tile([128, 2, BSH], bf16, tag="tr2",
                                         name="tr2")
                        nc.vector.tensor_tensor(tr2[:], tr1[:, 0:2, :],
                                                tr1[:, 2:4, :], OP.add)
                        nc.vector.tensor_tensor(coutb_sb[:, q, :], tr2[:, 0, :],
                                                tr2[:, 1, :], OP.add)
                        # gated residual + masked merge for this block
                        for bt in range(2):
                            csl = coutb_sb[:, q, bt * 128:(bt + 1) * 128]
                            og = psOG.tile([128, 2 * BS], f32, tag="og",
                                           name="og")
                            nc.tensor.matmul(og[:], csl, fgw_sb[:],
                                             start=True, stop=False)
                            nc.tensor.matmul(og[:], ones1_sb[:], fgb_sb[:],
                                             start=False, stop=True)
                            tano = pctmp.tile([128, BS], f32, tag="tano",
                                              name="tano")
                            nc.scalar.activation(tano[:], og[:, 0:BS], AF.Tanh)
                            sg = pctmp.tile([128, BS], f32, tag="sgx", name="sgx")
                            nc.scalar.activation(sg[:], og[:, BS:2 * BS],
                                                 AF.Sigmoid)
                            hatt = pctmp.tile([128, BS], f32, tag="hatt",
                                              name="hatt")
                            nc.vector.tensor_tensor(hatt[:], sg[:], tano[:],
                                                    OP.mult)
                            qsl = slice(q * BS, (q + 1) * BS)
                            # dh = d0 + hatt ; ho = mask*dh + hx
                            dh = pctmp.tile([128, BS], f32, tag="dhq", name="dhq")
                            nc.gpsimd.tensor_tensor(dh[:], hnew_sb[bt][:, qsl],
                                                    hatt[:], OP.add)
                            ho = pctmp.tile([128, BS], f32, tag="hoq", name="hoq",
                                            bufs=4)
                            nc.vector.scalar_tensor_tensor(ho[:], dh[:],
                                                           mask_sb[bt][:, q:q + 1],
                                                           hx_sb[bt][:, qsl],
                                                           OP.mult, OP.add)
                            nc.gpsimd.dma_start(
                                out=hx_out[bt * 128:(bt + 1) * 128, qsl],
                                in_=ho[:])

    _install_bir_fix(nc)
    return nc


# ---------------------------------------------------------------------------
# Host wrapper
# ---------------------------------------------------------------------------

def kernel(inp, hx, cx, wq_inp, wk_inp, wv_inp, w_ih, w_hh, b_ih, b_hh,
           wq_c, wk_c, wv_c, fc_w, fc_b, gate_w, gate_b, step=None):
    global last_exec_time_ns, last_results

    inp = np.asarray(inp, np.float32)
    hx = np.asarray(hx, np.float32)
    cx = np.asarray(cx, np.float32)
    wq_inp = np.asarray(wq_inp, np.float32)
    wk_inp = np.asarray(wk_inp, np.float32)
    wv_inp = np.asarray(wv_inp, np.float32)
    w_ih = np.asarray(w_ih, np.float32)
    w_hh = np.asarray(w_hh, np.float32)
    b_ih = np.asarray(b_ih, np.float32)
    b_hh = np.asarray(b_hh, np.float32)
    wq_c = np.asarray(wq_c, np.float32)
    wk_c = np.asarray(wk_c, np.float32)
    wv_c = np.asarray(wv_c, np.float32)
    fc_w = np.asarray(fc_w, np.float32)
    fc_b = np.asarray(fc_b, np.float32)
    gate_w = np.asarray(gate_w, np.float32)
    gate_b = np.asarray(gate_b, np.float32)

    if "nc" not in _CACHE:
        _CACHE["nc"] = _build()
    nc = _CACHE["nc"]

    # column permutations: per 256-wide hidden group g the fp8 panel holds
    # [i|o], the bf16 panel [f|g]  (torch gate order i,f,g,o)
    wcat = np.concatenate([w_ih.T, w_hh.T], axis=0)     # (4096, 8192)
    bias = (b_ih + b_hh)
    perm8 = np.concatenate([np.concatenate([
        np.arange(0 * NHID + g * BS, 0 * NHID + (g + 1) * BS),
        np.arange(3 * NHID + g * BS, 3 * NHID + (g + 1) * BS)])
        for g in range(8)])
    permbf = np.concatenate([np.concatenate([
        np.arange(1 * NHID + g * BS, 1 * NHID + (g + 1) * BS),
        np.arange(2 * NHID + g * BS, 2 * NHID + (g + 1) * BS)])
        for g in range(8)])
    w8_np = wcat[:, perm8].astype(E4)                   # (4096, 4096)
    wbf_np = wcat[:, permbf].astype(BF16)
    w8d = np.ascontiguousarray(w8_np.reshape(32, 128, 4096).transpose(1, 0, 2))
    wbfd = np.ascontiguousarray(wbf_np.reshape(32, 128, 4096).transpose(1, 0, 2))

    shared = {
        "wq": np.ascontiguousarray(
            wq_inp.reshape(NB, 2, 128, DKI).transpose(2, 1, 0, 3)),
        "wk1": np.ascontiguousarray(
            wk_inp[1].reshape(8, 128, DKI).transpose(1, 0, 2)),
        "wv1": np.ascontiguousarray(
            wv_inp[1].reshape(8, 128, BS).transpose(1, 0, 2)),
        "w8d": w8d,
        "wbfd": wbfd,
        "bias8": bias[perm8].astype(BF16).reshape(1, 4096),
        "biasbf": bias[permbf].astype(BF16).reshape(1, 4096),
        "wqc": np.ascontiguousarray(
            wq_c.astype(BF16).reshape(NB, 2, 128, 128).transpose(2, 1, 0, 3)),
        "wkc": np.ascontiguousarray(
            wk_c.astype(BF16).reshape(NB, 2, 128, 128).transpose(2, 1, 0, 3)),
        "wvc": np.ascontiguousarray(
            wv_c.astype(BF16).reshape(NB, 2, 128, 128).transpose(2, 1, 0, 3)),
        "fgw": np.ascontiguousarray(
            np.concatenate([fc_w, gate_w], axis=1)).astype(BF16),
        "fgb": np.concatenate([fc_b, gate_b]).astype(BF16).reshape(1, 2 * BS),
    }

    in_maps = []
    for c in range(NCORES):
        rs = slice(c * BSH, (c + 1) * BSH)
        inpT_c = inp[rs].T.reshape(8, 128, BSH).transpose(1, 0, 2)
        hxT = hx[rs].T.reshape(16, 128, BSH).transpose(1, 0, 2)
        m = {
            "inpT": np.ascontiguousarray(inpT_c),
            "hxT_f": np.ascontiguousarray(hxT),
            "hxT_b": np.ascontiguousarray(hxT.astype(BF16)),
            "hxT_8": np.ascontiguousarray(hxT.astype(E4)),
            "hx_bm": np.ascontiguousarray(hx[rs]),
            "cx_bm": np.ascontiguousarray(cx[rs]),
        }
        m.update(shared)
        in_maps.append(m)

    from concourse.bass_utils import run_bass_kernel_spmd
    trace = bool(int(os.environ.get("BASS_KTRACE", "0")))
    res = run_bass_kernel_spmd(nc, in_maps, list(range(NCORES)), trace=trace)
    last_exec_time_ns = res.exec_time_ns
    last_results = res

    hx_full = np.empty((B, NHID), np.float32)
    cx_full = np.empty((B, NHID), np.float32)
    mask_full = np.empty((B, NHID), np.float32)
    for c in range(NCORES):
        rs = slice(c * BSH, (c + 1) * BSH)
        hx_full[rs] = res.results[c]["hx_out"]
        cx_full[rs] = res.results[c]["cx_out"]
        mask_full[rs] = np.repeat(res.results[c]["mask_out"], BS, axis=1)
    return hx_full, cx_full, mask_full


# revision 19
# speedup vs baseline: 1.1709x; 1.0315x over previous
"""Trainium2 Bass kernel for nn_BlocksCore (RIMs BlocksCore step).

Data-parallel over batch B=2048 across 8 NeuronCores (256 rows each),
parameters replicated. Per-core plan (v3):

  A. input attention in f32 (mask-exact); inp_flat transposed to
     feature-major via DMA-transpose (bf16) + fp8 cast copies.
  B. LSTM gates: i,o via fp8e4 DoubleRow matmuls, f,g via bf16, processed
     per 256-wide hidden group (== one attention block); weight panels
     fetched two groups at a time ([128,2,1024] tiles) on the sync (fp8)
     and scalar (bf16) HWDGE queues; group tails (activations, c/h update,
     cx merge, h_new^T DMA-transpose) pipeline under the next group's
     matmuls. d0 = h_new - hx precomputed for the final merge.
  C. communication attention: q/k/v projections, one 32-row score tile for
     all (head, q-block) pairs, single softmax, PE-expanded apply with
     bf16 tree reductions, gated residual + masked merge per block.

Outputs: hx_out/cx_out [256,2048] f32, mask_out [256,8] (host expands).
"""

import json
import os

import numpy as np
import ml_dtypes

BF16 = ml_dtypes.bfloat16
E4 = ml_dtypes.float8_e4m3

B = 2048
NCORES = 8
BSH = B // NCORES          # 256 batch rows per core
NINP = 1024
NHID = 2048
NB = 8                     # blocks
BS = 256                   # block size (NHID / NB)
DKI = 64                   # input-attention d_k

_CACHE = {}
last_exec_time_ns = None
last_results = None

# ---------------------------------------------------------------------------
# BIR post-fix: this toolchain's core_v3 codegen supports only one sync-wait
# per CTRL-class instruction; hoist extras onto single-wait EventSemaphores.
# ---------------------------------------------------------------------------


def _fix_bir_json(bir_bytes: bytes) -> bytes:
    bir = json.loads(bir_bytes)
    for fn in bir.get("functions", []):
        for blk in fn.get("blocks", []):
            out = []
            for ins in blk.get("instructions", []):
                si = ins.get("sync_info") or {}
                waits = si.get("on_wait") or []
                if len(waits) > 1:
                    for j, w in enumerate(waits[:-1]):
                        out.append({
                            "name": f"{ins['name']}-w{j}",
                            "engine": ins["engine"],
                            "opcode": "EventSemaphore",
                            "ins": [],
                            "outs": [],
                            "sync_info": {"on_update": [], "on_wait": [w]},
                        })
                    si = dict(si)
                    si["on_wait"] = [waits[-1]]
                    ins = dict(ins)
                    ins["sync_info"] = si
                out.append(ins)
            blk["instructions"] = out
    return json.dumps(bir).encode()


def _install_bir_fix(nc):
    orig = nc.to_json_bytes

    def patched(*a, **k):
        return _fix_bir_json(orig(*a, **k))

    nc.to_json_bytes = patched


# ---------------------------------------------------------------------------
# Device kernel
# ---------------------------------------------------------------------------

def _build():
    import concourse.bass as bass
    import concourse.tile as tile
    from concourse import mybir

    f32 = mybir.dt.float32
    bf16 = mybir.dt.bfloat16
    fp8 = mybir.dt.float8e4
    OP = mybir.AluOpType
    AF = mybir.ActivationFunctionType
    AX = mybir.AxisListType
    DR = mybir.MatmulPerfMode.DoubleRow

    nc = bass.Bass()

    # ---- I/O ------------------------------------------------------------
    inpT = nc.declare_dram_parameter("inpT", [128, 8, BSH], f32, isOutput=False)
    hxT_f = nc.declare_dram_parameter("hxT_f", [128, 16, BSH], f32, isOutput=False)
    hxT_b = nc.declare_dram_parameter("hxT_b", [128, 16, BSH], bf16, isOutput=False)
    hxT_8 = nc.declare_dram_parameter("hxT_8", [128, 16, BSH], fp8, isOutput=False)
    hx_bm = nc.declare_dram_parameter("hx_bm", [BSH, NHID], f32, isOutput=False)
    cx_bm = nc.declare_dram_parameter("cx_bm", [BSH, NHID], f32, isOutput=False)
    wq = nc.declare_dram_parameter("wq", [128, 2, NB, DKI], f32, isOutput=False)
    wk1 = nc.declare_dram_parameter("wk1", [128, 8, DKI], f32, isOutput=False)
    wv1 = nc.declare_dram_parameter("wv1", [128, 8, BS], f32, isOutput=False)
    # LSTM weights: [128, 32 ktiles, 8 groups * 512] — per group g the fp8
    # panel holds [i|o] columns for hidden chunk g, the bf16 panel [f|g].
    w8d = nc.declare_dram_parameter("w8d", [128, 32, 4096], fp8, isOutput=False)
    wbfd = nc.declare_dram_parameter("wbfd", [128, 32, 4096], bf16, isOutput=False)
    bias8 = nc.declare_dram_parameter("bias8", [1, 4096], bf16, isOutput=False)
    biasbf = nc.declare_dram_parameter("biasbf", [1, 4096], bf16, isOutput=False)
    wqc = nc.declare_dram_parameter("wqc", [128, 2, NB, 128], bf16, isOutput=False)
    wkc = nc.declare_dram_parameter("wkc", [128, 2, NB, 128], bf16, isOutput=False)
    wvc = nc.declare_dram_parameter("wvc", [128, 2, NB, 128], bf16, isOutput=False)
    fgw = nc.declare_dram_parameter("fgw", [128, 2 * BS], bf16, isOutput=False)
    fgb = nc.declare_dram_parameter("fgb", [1, 2 * BS], bf16, isOutput=False)
    hx_out = nc.declare_dram_parameter("hx_out", [BSH, NHID], f32, isOutput=True)
    cx_out = nc.declare_dram_parameter("cx_out", [BSH, NHID], f32, isOutput=True)
    mask_out = nc.declare_dram_parameter("mask_out", [BSH, NB], f32, isOutput=True)

    # ---- inline constants ----------------------------------------------
    hq_np = np.zeros((128, NB, 32), dtype=BF16)
    for d in range(128):
        for q in range(NB):
            hq_np[d, q, (d // 32) * 8 + q] = 1
    e32_np = np.zeros((32, NB, 128), dtype=BF16)
    for m in range(128):
        for q in range(NB):
            e32_np[(m // 32) * 8 + q, q, m] = 1
    # partition broadcaster: sel8[n', n, p] = (n' == n); a K=8 matmul with
    # lhsT=sel8[:, n, :] replicates row n of the rhs across 128 partitions
    sel8_np = np.zeros((8, NB, 128), dtype=BF16)
    for n in range(NB):
        sel8_np[n, n, :] = 1
    hqc = nc.inline_tensor(hq_np, "hqc")
    e32b = nc.inline_tensor(e32_np, "e32b")
    ones1c = nc.inline_tensor(np.ones((1, 128), dtype=BF16), "ones1c")
    sel8c = nc.inline_tensor(sel8_np, "sel8c")
    identc = nc.inline_tensor(np.eye(128, dtype=BF16), "identc")

    with tile.TileContext(nc) as tc:
        with tc.tile_pool(name="cp", bufs=1) as cp, \
             tc.tile_pool(name="pp", bufs=1) as pp:
            # fast-path inputs on sync (needed within ~5us)
            bias8_sb = cp.tile([1, 4096], bf16)
            nc.sync.dma_start(out=bias8_sb[:], in_=bias8[:])
            hxT8_sb = pp.tile([128, 16, BSH], fp8)
            hxTb_sb = pp.tile([128, 16, BSH], bf16)

            # constants and late inputs on gpsimd
            hq_sb = cp.tile([128, NB, 32], bf16)
            nc.gpsimd.dma_start(out=hq_sb[:], in_=hqc[:])
            e32_sb = cp.tile([32, NB, 128], bf16)
            nc.gpsimd.dma_start(out=e32_sb[:], in_=e32b[:])
            ones1_sb = cp.tile([1, 128], bf16)
            nc.gpsimd.dma_start(out=ones1_sb[:], in_=ones1c[:])
            fgw_sb = cp.tile([128, 2 * BS], bf16)
            nc.gpsimd.dma_start(out=fgw_sb[:], in_=fgw[:])
            fgb_sb = cp.tile([1, 2 * BS], bf16)
            nc.gpsimd.dma_start(out=fgb_sb[:], in_=fgb[:])
            wqc_sb = cp.tile([128, 2, NB, 128], bf16)
            nc.gpsimd.dma_start(out=wqc_sb[:], in_=wqc[:])
            wkc_sb = cp.tile([128, 2, NB, 128], bf16)
            nc.gpsimd.dma_start(out=wkc_sb[:], in_=wkc[:])
            wvc_sb = cp.tile([128, 2, NB, 128], bf16)
            nc.gpsimd.dma_start(out=wvc_sb[:], in_=wvc[:])
            cx_sb = [pp.tile([128, NHID], f32, tag=f"cx{bt}", name=f"cx{bt}")
                     for bt in range(2)]
            for bt in range(2):
                nc.gpsimd.dma_start(out=cx_sb[bt][:],
                                    in_=cx_bm[bt * 128:(bt + 1) * 128, :])
            hx_sb = [pp.tile([128, NHID], f32, tag=f"hx{bt}", name=f"hx{bt}")
                     for bt in range(2)]
            for bt in range(2):
                nc.gpsimd.dma_start(out=hx_sb[bt][:],
                                    in_=hx_bm[bt * 128:(bt + 1) * 128, :])

            sel8_sb = cp.tile([8, NB, 128], bf16)
            nc.gpsimd.dma_start(out=sel8_sb[:], in_=sel8c[:])
            ident_sb = cp.tile([128, 128], bf16)
            nc.gpsimd.dma_start(out=ident_sb[:], in_=identc[:])
            xt8_sb = pp.tile([128, 16, BSH], fp8)
            xtb_sb = pp.tile([128, 16, BSH], bf16)
            # hnew_sb holds h_new per group, overwritten in place by
            # d0 = h_new - hx once hnb/hnewT snapshots are taken
            hnew_sb = [pp.tile([128, NHID], f32, tag=f"hn{bt}", name=f"hn{bt}")
                       for bt in range(2)]
            hnewT_sb = pp.tile([128, 16, BSH], bf16)
            mask_sb = [pp.tile([128, NB], f32, tag=f"mk{bt}", name=f"mk{bt}")
                       for bt in range(2)]
            sig_sb = [pp.tile([128, NB], bf16, tag=f"sg{bt}", name=f"sg{bt}")
                      for bt in range(2)]
            qc_sb = pp.tile([128, NB, BSH], bf16)
            kc_sb = pp.tile([128, NB, BSH], bf16)
            vc_sb = pp.tile([128, NB, BSH], bf16)

            # ---- phase A (f32, mask-exact) -------------------------------
            with tc.tile_pool(name="pa", bufs=1) as pa, \
                 tc.tile_pool(name="pa2", bufs=2) as pa2, \
                 tc.tile_pool(name="paps", bufs=1, space="PSUM") as paps:
                # A inputs first on the scalar queue (arrive ~3us)
                inpT_sb = pa.tile([128, 8, BSH], f32)
                nc.sync.dma_start(out=inpT_sb[:], in_=inpT[:])
                wk1_sb = pa.tile([128, 8, DKI], f32)
                nc.sync.dma_start(out=wk1_sb[:], in_=wk1[:])
                wv1_sb = pa.tile([128, 8, BS], f32)
                nc.sync.dma_start(out=wv1_sb[:], in_=wv1[:])
                nc.sync.dma_start(out=hxT8_sb[:], in_=hxT_8[:])
                nc.sync.dma_start(out=hxTb_sb[:], in_=hxT_b[:])
                wq_sb = pa.tile([128, 2, NB, DKI], f32)
                nc.scalar.dma_start(out=wq_sb[:], in_=wq[:])
                hxTf_sb = pa.tile([128, 16, BSH], f32)
                nc.scalar.dma_start(out=hxTf_sb[:], in_=hxT_f[:])
                biasbf_sb = cp.tile([1, 4096], bf16)
                nc.scalar.dma_start(out=biasbf_sb[:], in_=biasbf[:])

                # v1^T = wv1^T @ inp^T directly in feature-major (both halves)
                v1T_sb = pa.tile([128, 2, BSH], f32)
                for s in range(2):
                    v1T_ps = paps.tile([128, BSH], f32, tag="v1T")
                    for k in range(8):
                        nc.tensor.matmul(v1T_ps[:],
                                         wv1_sb[:, k, s * 128:(s + 1) * 128],
                                         inpT_sb[:, k, :],
                                         start=(k == 0), stop=(k == 7))
                    nc.vector.tensor_copy(v1T_sb[:, s, :], v1T_ps[:])

                sigT_sb = pa.tile([8, BSH], bf16)
                for bt in range(2):
                    bsl = slice(bt * 128, (bt + 1) * 128)
                    k1_ps = paps.tile([128, DKI], f32, tag="k1")
                    for k in range(8):
                        nc.tensor.matmul(k1_ps[:], inpT_sb[:, k, bsl],
                                         wk1_sb[:, k, :],
                                         start=(k == 0), stop=(k == 7))
                    k1s = pa2.tile([128, DKI], f32, tag="k1s")
                    nc.vector.tensor_copy(k1s[:], k1_ps[:])

                    q_ps = paps.tile([128, NB, DKI], f32, tag="q")
                    for n in range(NB):
                        for s in range(2):
                            nc.tensor.matmul(q_ps[:, n, :],
                                             hxTf_sb[:, 2 * n + s, bsl],
                                             wq_sb[:, s, n, :],
                                             start=(s == 0), stop=(s == 1))
                    prod = pa2.tile([128, NB, DKI], f32, tag="prod")
                    k1a = k1s[:]
                    k1bc = bass.AP(tensor=k1a.tensor, offset=k1a.offset,
                                   ap=[k1a.ap[0], [0, NB], k1a.ap[1]])
                    nc.vector.tensor_tensor(prod[:], q_ps[:], k1bc, OP.mult)
                    s1 = pa2.tile([128, NB], f32, tag="s1")
                    nc.vector.reduce_sum(s1[:], prod[:], axis=AX.X)
                    nc.scalar.activation(sig_sb[bt][:], s1[:], AF.Sigmoid,
                                         scale=0.125)

                    # top-4 mask (rank counts fused via accum_out)
                    cnt = pa2.tile([128, NB], f32, tag="cnt")
                    tmp = pa2.tile([128, NB], f32, tag="tmp")
                    for n in range(NB):
                        nc.vector.tensor_scalar(tmp[:], s1[:], s1[:, n:n + 1],
                                                0.0, OP.is_gt, OP.add,
                                                accum_out=cnt[:, n:n + 1])
                    nc.vector.tensor_single_scalar(mask_sb[bt][:], cnt[:], 4.0,
                                                   OP.is_lt)
                    nc.gpsimd.dma_start(out=mask_out[bsl, :], in_=mask_sb[bt][:])
                    # sig^T half for the partition broadcast below
                    sgt = paps.tile([8, 128], bf16, tag="sgt")
                    nc.tensor.transpose(sgt[:], sig_sb[bt][:], ident_sb[:])
                    nc.vector.tensor_copy(sigT_sb[:, bsl], sgt[:])

                # inp_flat^T = v1^T * broadcast(sig^T) per block, cast to
                # bf16 (f,g path) and fp8 (DoubleRow path)
                with tc.tile_pool(name="pasg", bufs=2, space="PSUM") as pasg:
                    for n in range(NB):
                        sgb = pasg.tile([128, BSH], f32, tag="sgb")
                        nc.tensor.matmul(sgb[:], sel8_sb[:, n, :], sigT_sb[:],
                                         start=True, stop=True)
                        for s in range(2):
                            nc.vector.tensor_tensor(xtb_sb[:, 2 * n + s, :],
                                                    v1T_sb[:, s, :], sgb[:],
                                                    OP.mult)
                            nc.vector.tensor_tensor(xt8_sb[:, 2 * n + s, :],
                                                    v1T_sb[:, s, :], sgb[:],
                                                    OP.mult)

            # ---- phase B: LSTM groups, two per weight fetch --------------
            pair_order = list(range(8, 16)) + list(range(8))
            with tc.tile_pool(name="gps", bufs=1, space="PSUM") as gps, \
                 tc.tile_pool(name="pw", bufs=8) as pw, \
                 tc.tile_pool(name="pb2", bufs=2) as pb2:
                for gpair in range(4):
                    g_all = {}
                    for sub in range(2):
                        for bt in range(2):
                            g_all[sub, bt] = gps.tile(
                                [128, 4, BS], f32, tag=f"g{sub}{bt}",
                                name=f"g{sub}{bt}")
                    csl2 = slice(gpair * 1024, (gpair + 1) * 1024)
                    for sub in range(2):
                        gq = 2 * gpair + sub
                        csl = slice(gq * 512, (gq + 1) * 512)
                        for bt in range(2):
                            nc.tensor.matmul(g_all[sub, bt][:, 0:2, :],
                                             ones1_sb[:], bias8_sb[:, csl],
                                             start=True, stop=False)
                            nc.tensor.matmul(g_all[sub, bt][:, 2:4, :],
                                             ones1_sb[:], biasbf_sb[:, csl],
                                             start=True, stop=False)
                    for j in pair_order:
                        w8t = pw.tile([128, 2, 1024], fp8, tag="w8t")
                        nc.sync.dma_start(out=w8t[:],
                                          in_=w8d[:, 2 * j:2 * j + 2, csl2])
                        wbt = pw.tile([128, 2, 1024], bf16, tag="wbt")
                        nc.sync.dma_start(out=wbt[:],
                                          in_=wbfd[:, 2 * j:2 * j + 2, csl2])
                        st = (j == pair_order[-1])
                        for sub in range(2):
                            off = sub * 512
                            for bt in range(2):
                                bsl = slice(bt * 128, (bt + 1) * 128)
                                if j >= 8:
                                    t = 2 * (j - 8)
                                    lhs8 = [hxT8_sb[:, t + kk, bsl]
                                            for kk in range(2)]
                                    lhsb = [hxTb_sb[:, t + kk, bsl]
                                            for kk in range(2)]
                                else:
                                    lhs8 = [xt8_sb[:, 2 * j + kk, bsl]
                                            for kk in range(2)]
                                    lhsb = [xtb_sb[:, 2 * j + kk, bsl]
                                            for kk in range(2)]
                                for kk in range(2):
                                    nc.tensor.matmul(
                                        g_all[sub, bt][:, 0:2, :], lhs8[kk],
                                        w8t[:, kk, off:off + 512],
                                        start=False, stop=(st and kk == 1))
                                    nc.tensor.matmul(
                                        g_all[sub, bt][:, 2:4, :], lhsb[kk],
                                        wbt[:, kk, off:off + 512],
                                        start=False, stop=(st and kk == 1))
                    # ---- group tails ----------------------------------
                    # PSUM-freeing activation reads first for all four
                    # (sub, bt) so the next group-pair's matmuls unblock
                    # after ~3 ACT ops instead of a full tail chain.
                    acts = {}
                    for sub in range(2):
                        for bt in range(2):
                            sio = pb2.tile([128, 2, BS], f32, tag="sio",
                                           name="sio", bufs=4)
                            nc.scalar.activation(sio[:], g_all[sub, bt][:, 0:2, :],
                                                 AF.Sigmoid)
                            sigf = pb2.tile([128, BS], f32, tag="sigf",
                                            name="sigf", bufs=4)
                            nc.scalar.activation(sigf[:], g_all[sub, bt][:, 2, :],
                                                 AF.Sigmoid)
                            tang = pb2.tile([128, BS], f32, tag="tang",
                                            name="tang", bufs=4)
                            nc.scalar.activation(tang[:], g_all[sub, bt][:, 3, :],
                                                 AF.Tanh)
                            acts[sub, bt] = (sio, sigf, tang)
                    for sub in range(2):
                        gq = 2 * gpair + sub
                        sl = slice(gq * BS, (gq + 1) * BS)
                        for bt in range(2):
                            sio, sigf, tang = acts[sub, bt]
                            t1 = pb2.tile([128, BS], f32, tag="t1", name="t1")
                            nc.vector.tensor_tensor(t1[:], sigf[:],
                                                    cx_sb[bt][:, sl], OP.mult)
                            t2 = pb2.tile([128, BS], f32, tag="t2", name="t2")
                            nc.gpsimd.tensor_tensor(t2[:], sio[:, 0, :], tang[:],
                                                    OP.mult)
                            cnew = pb2.tile([128, BS], f32, tag="cnew",
                                            name="cnew")
                            nc.vector.tensor_tensor(cnew[:], t1[:], t2[:], OP.add)
                            t3 = pb2.tile([128, BS], f32, tag="t3", name="t3")
                            nc.scalar.activation(t3[:], cnew[:], AF.Tanh)
                            nc.vector.tensor_tensor(hnew_sb[bt][:, sl],
                                                    sio[:, 1, :], t3[:], OP.mult)
                            hnb = pb2.tile([128, BS], bf16, tag="hnb", name="hnb")
                            nc.vector.tensor_copy(hnb[:], hnew_sb[bt][:, sl])
                            dc = pb2.tile([128, BS], f32, tag="dc", name="dc")
                            nc.gpsimd.tensor_tensor(dc[:], cnew[:],
                                                    cx_sb[bt][:, sl], OP.subtract)
                            co = pb2.tile([128, BS], f32, tag="co", name="co")
                            nc.vector.scalar_tensor_tensor(
                                co[:], dc[:], mask_sb[bt][:, gq:gq + 1],
                                cx_sb[bt][:, sl], OP.mult, OP.add)
                            nc.gpsimd.dma_start(
                                out=cx_out[bt * 128:(bt + 1) * 128, sl],
                                in_=co[:])
                            for s in range(2):
                                nc.scalar.dma_start(
                                    out=hnewT_sb[:, 2 * gq + s,
                                                 bt * 128:(bt + 1) * 128],
                                    in_=hnb[:, s * 128:(s + 1) * 128],
                                    transpose=True)
                            # d0 = h_new - hx, in place (merge shortcut)
                            nc.gpsimd.tensor_tensor(hnew_sb[bt][:, sl],
                                                    hnew_sb[bt][:, sl],
                                                    hx_sb[bt][:, sl],
                                                    OP.subtract)

            # ============================ phase C ========================
            with tc.tile_pool(name="pc", bufs=1) as pc, \
                 tc.tile_pool(name="pctmp", bufs=3) as pctmp:
                at_sb = pc.tile([32, NB, BSH], bf16)
                coutb_sb = pc.tile([128, NB, BSH], bf16)
                with tc.tile_pool(name="psS", bufs=1, space="PSUM") as psS, \
                     tc.tile_pool(name="prjC", bufs=2, space="PSUM") as prj:
                    for wsb, dst in ((wkc_sb, kc_sb), (wvc_sb, vc_sb)):
                        for n in range(NB):
                            ps = prj.tile([128, BSH], f32, tag="proj")
                            for s in range(2):
                                nc.tensor.matmul(ps[:], wsb[:, s, n, :],
                                                 hnewT_sb[:, 2 * n + s, :],
                                                 start=(s == 0), stop=(s == 1))
                            nc.scalar.copy(dst[:, n, :], ps[:])
                    S = psS.tile([32, NB, BSH], f32, tag="S", name="S")
                    for q in range(NB):
                        ps = prj.tile([128, BSH], f32, tag="proj")
                        for s in range(2):
                            nc.tensor.matmul(ps[:], wqc_sb[:, s, q, :],
                                             hnewT_sb[:, 2 * q + s, :],
                                             start=(s == 0), stop=(s == 1))
                        nc.scalar.copy(qc_sb[:, q, :], ps[:])
                        pr = pctmp.tile([128, NB, BSH], bf16, tag="pr", name="pr")
                        qa = qc_sb[:, q, :]
                        qbc = bass.AP(tensor=qa.tensor, offset=qa.offset,
                                      ap=[qa.ap[0], [0, NB], qa.ap[-1]])
                        nc.vector.tensor_tensor(pr[:], qbc, kc_sb[:], OP.mult)
                        for kp in range(4):
                            nc.tensor.matmul(S[:, 2 * kp:2 * kp + 2, :],
                                             hq_sb[:, q, :],
                                             pr[:, 2 * kp:2 * kp + 2, :],
                                             start=(q == 0), stop=(q == 7))
                    ex = pc.tile([32, NB, BSH], bf16, tag="ex", name="ex")
                    nc.scalar.activation(ex[:], S[:], AF.Exp,
                                         scale=float(1.0 / np.sqrt(32.0)))
                    # denominator by bf16 tree adds (contiguous slices)
                    e1 = pctmp.tile([32, 4, BSH], bf16, tag="e1", name="e1")
                    nc.vector.tensor_tensor(e1[:], ex[:, 0:4, :], ex[:, 4:8, :],
                                            OP.add)
                    e2 = pctmp.tile([32, 2, BSH], bf16, tag="e2", name="e2")
                    nc.vector.tensor_tensor(e2[:], e1[:, 0:2, :], e1[:, 2:4, :],
                                            OP.add)
                    denom = pctmp.tile([32, BSH], f32, tag="denom", name="denom")
                    nc.vector.tensor_tensor(denom[:], e2[:, 0, :], e2[:, 1, :],
                                            OP.add)
                    recip = pctmp.tile([32, BSH], f32, tag="recip", name="recip")
                    nc.vector.reciprocal(recip[:], denom[:])
                    ra = recip[:]
                    rbc = bass.AP(tensor=ra.tensor, offset=ra.offset,
                                  ap=[ra.ap[0], [0, NB], ra.ap[-1]])
                    nc.vector.tensor_tensor(at_sb[:], ex[:], rbc, OP.mult)

                with tc.tile_pool(name="psU", bufs=1, space="PSUM") as psU, \
                     tc.tile_pool(name="psOG", bufs=2, space="PSUM") as psOG:
                    for q in range(NB):
                        U = psU.tile([128, NB, BSH], f32, tag="U", name="U")
                        for kp in range(4):
                            nc.tensor.matmul(U[:, 2 * kp:2 * kp + 2, :],
                                             e32_sb[:, q, :],
                                             at_sb[:, 2 * kp:2 * kp + 2, :],
                                             start=True, stop=True)
                        prods = pctmp.tile([128, NB, BSH], bf16, tag="prods")
                        nc.vector.tensor_tensor(prods[:], U[:], vc_sb[:], OP.mult)
                        tr1 = pctmp.tile([128, 4, BSH], bf16, tag="tr1",
                                         name="tr1")
                        nc.vector.tensor_tensor(tr1[:], prods[:, 0:4, :],
                                                prods[:, 4:8, :], OP.add)
                        tr2 = pctmp.tile([128, 2, BSH], bf16, tag="tr2",
                                         name="tr2")
                        nc.vector.tensor_tensor(tr2[:], tr1[:, 0:2, :],
                                                tr1[:, 2:4, :], OP.add)
                        nc.vector.tensor_tensor(coutb_sb[:, q, :], tr2[:, 0, :],
                                                tr2[:, 1, :], OP.add)
                        # gated residual + masked merge for this block
                        for bt in range(2):
                            csl = coutb_sb[:, q, bt * 128:(bt + 1) * 128]
                            og = psOG.tile([128, 2 * BS], f32, tag="og",
                                           name="og")
                            nc.tensor.matmul(og[:], csl, fgw_sb[:],
                                             start=True, stop=False)
                            nc.tensor.matmul(og[:], ones1_sb[:], fgb_sb[:],
                                             start=False, stop=True)
                            tano = pctmp.tile([128, BS], f32, tag="tano",
                                              name="tano")
                            nc.scalar.activation(tano[:], og[:, 0:BS], AF.Tanh)
                            sg = pctmp.tile([128, BS], f32, tag="sgx", name="sgx")
                            nc.scalar.activation(sg[:], og[:, BS:2 * BS],
                                                 AF.Sigmoid)
                            hatt = pctmp.tile([128, BS], f32, tag="hatt",
                                              name="hatt")
                            nc.vector.tensor_tensor(hatt[:], sg[:], tano[:],
                                                    OP.mult)
                            qsl = slice(q * BS, (q + 1) * BS)
                            # dh = d0 + hatt ; ho = mask*dh + hx
                            dh = pctmp.tile([128, BS], f32, tag="dhq", name="dhq")
                            nc.gpsimd.tensor_tensor(dh[:], hnew_sb[bt][:, qsl],
                                                    hatt[:], OP.add)
                            ho = pctmp.tile([128, BS], f32, tag="hoq", name="hoq",
                                            bufs=4)
                            nc.vector.scalar_tensor_tensor(ho[:], dh[:],
                                                           mask_sb[bt][:, q:q + 1],
                                                           hx_sb[bt][:, qsl],
                                                           OP.mult, OP.add)
                            nc.gpsimd.dma_start(
                                out=hx_out[bt * 128:(bt + 1) * 128, qsl],
                                in_=ho[:])

    _install_bir_fix(nc)
    return nc


# ---------------------------------------------------------------------------
# Host wrapper
# ---------------------------------------------------------------------------

def kernel(inp, hx, cx, wq_inp, wk_inp, wv_inp, w_ih, w_hh, b_ih, b_hh,
           wq_c, wk_c, wv_c, fc_w, fc_b, gate_w, gate_b, step=None):
    global last_exec_time_ns, last_results

    inp = np.asarray(inp, np.float32)
    hx = np.asarray(hx, np.float32)
    cx = np.asarray(cx, np.float32)
    wq_inp = np.asarray(wq_inp, np.float32)
    wk_inp = np.asarray(wk_inp, np.float32)
    wv_inp = np.asarray(wv_inp, np.float32)
    w_ih = np.asarray(w_ih, np.float32)
    w_hh = np.asarray(w_hh, np.float32)
    b_ih = np.asarray(b_ih, np.float32)
    b_hh = np.asarray(b_hh, np.float32)
    wq_c = np.asarray(wq_c, np.float32)
    wk_c = np.asarray(wk_c, np.float32)
    wv_c = np.asarray(wv_c, np.float32)
    fc_w = np.asarray(fc_w, np.float32)
    fc_b = np.asarray(fc_b, np.float32)
    gate_w = np.asarray(gate_w, np.float32)
    gate_b = np.asarray(gate_b, np.float32)

    if "nc" not in _CACHE:
        _CACHE["nc"] = _build()
    nc = _CACHE["nc"]

    # column permutations: per 256-wide hidden group g the fp8 panel holds
    # [i|o], the bf16 panel [f|g]  (torch gate order i,f,g,o)
    wcat = np.concatenate([w_ih.T, w_hh.T], axis=0)     # (4096, 8192)
    bias = (b_ih + b_hh)
    perm8 = np.concatenate([np.concatenate([
        np.arange(0 * NHID + g * BS, 0 * NHID + (g + 1) * BS),
        np.arange(3 * NHID + g * BS, 3 * NHID + (g + 1) * BS)])
        for g in range(8)])
    permbf = np.concatenate([np.concatenate([
        np.arange(1 * NHID + g * BS, 1 * NHID + (g + 1) * BS),
        np.arange(2 * NHID + g * BS, 2 * NHID + (g + 1) * BS)])
        for g in range(8)])
    w8_np = wcat[:, perm8].astype(E4)                   # (4096, 4096)
    wbf_np = wcat[:, permbf].astype(BF16)
    w8d = np.ascontiguousarray(w8_np.reshape(32, 128, 4096).transpose(1, 0, 2))
    wbfd = np.ascontiguousarray(wbf_np.reshape(32, 128, 4096).transpose(1, 0, 2))

    shared = {
        "wq": np.ascontiguousarray(
            wq_inp.reshape(NB, 2, 128, DKI).transpose(2, 1, 0, 3)),
        "wk1": np.ascontiguousarray(
            wk_inp[1].reshape(8, 128, DKI).transpose(1, 0, 2)),
        "wv1": np.ascontiguousarray(
            wv_inp[1].reshape(8, 128, BS).transpose(1, 0, 2)),
        "w8d": w8d,
        "wbfd": wbfd,
        "bias8": bias[perm8].astype(BF16).reshape(1, 4096),
        "biasbf": bias[permbf].astype(BF16).reshape(1, 4096),
        "wqc": np.ascontiguousarray(
            wq_c.astype(BF16).reshape(NB, 2, 128, 128).transpose(2, 1, 0, 3)),
        "wkc": np.ascontiguousarray(
            wk_c.astype(BF16).reshape(NB, 2, 128, 128).transpose(2, 1, 0, 3)),
        "wvc": np.ascontiguousarray(
            wv_c.astype(BF16).reshape(NB, 2, 128, 128).transpose(2, 1, 0, 3)),
        "fgw": np.ascontiguousarray(
            np.concatenate([fc_w, gate_w], axis=1)).astype(BF16),
        "fgb": np.concatenate([fc_b, gate_b]).astype(BF16).reshape(1, 2 * BS),
    }

    in_maps = []
    for c in range(NCORES):
        rs = slice(c * BSH, (c + 1) * BSH)
        inpT_c = inp[rs].T.reshape(8, 128, BSH).transpose(1, 0, 2)
        hxT = hx[rs].T.reshape(16, 128, BSH).transpose(1, 0, 2)
        m = {
            "inpT": np.ascontiguousarray(inpT_c),
            "hxT_f": np.ascontiguousarray(hxT),
            "hxT_b": np.ascontiguousarray(hxT.astype(BF16)),
            "hxT_8": np.ascontiguousarray(hxT.astype(E4)),
            "hx_bm": np.ascontiguousarray(hx[rs]),
            "cx_bm": np.ascontiguousarray(cx[rs]),
        }
        m.update(shared)
        in_maps.append(m)

    from concourse.bass_utils import run_bass_kernel_spmd
    trace = bool(int(os.environ.get("BASS_KTRACE", "0")))
    res = run_bass_kernel_spmd(nc, in_maps, list(range(NCORES)), trace=trace)
    last_exec_time_ns = res.exec_time_ns
    last_results = res

    hx_full = np.empty((B, NHID), np.float32)
    cx_full = np.empty((B, NHID), np.float32)
    mask_full = np.empty((B, NHID), np.float32)
    for c in range(NCORES):
        rs = slice(c * BSH, (c + 1) * BSH)
        hx_full[rs] = res.results[c]["hx_out"]
        cx_full[rs] = res.results[c]["cx_out"]
        mask_full[rs] = np.repeat(res.results[c]["mask_out"], BS, axis=1)
    return hx_full, cx_full, mask_full


# revision 20
# speedup vs baseline: 1.1867x; 1.0135x over previous
"""Trainium2 Bass kernel for nn_BlocksCore (RIMs BlocksCore step).

Data-parallel over batch B=2048 across 8 NeuronCores (256 rows each),
parameters replicated. Per-core plan (v3):

  A. input attention in f32 (mask-exact); inp_flat transposed to
     feature-major via DMA-transpose (bf16) + fp8 cast copies.
  B. LSTM gates: i,o via fp8e4 DoubleRow matmuls, f,g via bf16, processed
     per 256-wide hidden group (== one attention block); weight panels
     fetched two groups at a time ([128,2,1024] tiles) on the sync (fp8)
     and scalar (bf16) HWDGE queues; group tails (activations, c/h update,
     cx merge, h_new^T DMA-transpose) pipeline under the next group's
     matmuls. d0 = h_new - hx precomputed for the final merge.
  C. communication attention: q/k/v projections, one 32-row score tile for
     all (head, q-block) pairs, single softmax, PE-expanded apply with
     bf16 tree reductions, gated residual + masked merge per block.

Outputs: hx_out/cx_out [256,2048] f32, mask_out [256,8] (host expands).
"""

import json
import os

import numpy as np
import ml_dtypes

BF16 = ml_dtypes.bfloat16
E4 = ml_dtypes.float8_e4m3

B = 2048
NCORES = 8
BSH = B // NCORES          # 256 batch rows per core
NINP = 1024
NHID = 2048
NB = 8                     # blocks
BS = 256                   # block size (NHID / NB)
DKI = 64                   # input-attention d_k

_CACHE = {}
last_exec_time_ns = None
last_results = None

# ---------------------------------------------------------------------------
# BIR post-fix: this toolchain's core_v3 codegen supports only one sync-wait
# per CTRL-class instruction; hoist extras onto single-wait EventSemaphores.
# ---------------------------------------------------------------------------


def _fix_bir_json(bir_bytes: bytes) -> bytes:
    bir = json.loads(bir_bytes)
    for fn in bir.get("functions", []):
        for blk in fn.get("blocks", []):
            out = []
            for ins in blk.get("instructions", []):
                si = ins.get("sync_info") or {}
                waits = si.get("on_wait") or []
                if len(waits) > 1:
                    for j, w in enumerate(waits[:-1]):
                        out.append({
                            "name": f"{ins['name']}-w{j}",
                            "engine": ins["engine"],
                            "opcode": "EventSemaphore",
                            "ins": [],
                            "outs": [],
                            "sync_info": {"on_update": [], "on_wait": [w]},
                        })
                    si = dict(si)
                    si["on_wait"] = [waits[-1]]
                    ins = dict(ins)
                    ins["sync_info"] = si
                out.append(ins)
            blk["instructions"] = out
    return json.dumps(bir).encode()


def _install_bir_fix(nc):
    orig = nc.to_json_bytes

    def patched(*a, **k):
        return _fix_bir_json(orig(*a, **k))

    nc.to_json_bytes = patched


# ---------------------------------------------------------------------------
# Device kernel
# ---------------------------------------------------------------------------

def _build():
    import concourse.bass as bass
    import concourse.tile as tile
    from concourse import mybir

    f32 = mybir.dt.float32
    bf16 = mybir.dt.bfloat16
    fp8 = mybir.dt.float8e4
    OP = mybir.AluOpType
    AF = mybir.ActivationFunctionType
    AX = mybir.AxisListType
    DR = mybir.MatmulPerfMode.DoubleRow

    nc = bass.Bass()

    # ---- I/O ------------------------------------------------------------
    inpT = nc.declare_dram_parameter("inpT", [128, 8, BSH], f32, isOutput=False)
    hxT_f = nc.declare_dram_parameter("hxT_f", [128, 16, BSH], f32, isOutput=False)
    hxT_b = nc.declare_dram_parameter("hxT_b", [128, 16, BSH], bf16, isOutput=False)
    hxT_8 = nc.declare_dram_parameter("hxT_8", [128, 16, BSH], fp8, isOutput=False)
    hx_bm = nc.declare_dram_parameter("hx_bm", [BSH, NHID], f32, isOutput=False)
    cx_bm = nc.declare_dram_parameter("cx_bm", [BSH, NHID], f32, isOutput=False)
    wq = nc.declare_dram_parameter("wq", [128, 2, NB, DKI], f32, isOutput=False)
    wk1 = nc.declare_dram_parameter("wk1", [128, 8, DKI], f32, isOutput=False)
    wv1 = nc.declare_dram_parameter("wv1", [128, 8, BS], f32, isOutput=False)
    # LSTM weights: [128, 32 ktiles, 8 groups * 512] — per group g the fp8
    # panel holds [i|o] columns for hidden chunk g, the bf16 panel [f|g].
    w8d = nc.declare_dram_parameter("w8d", [128, 32, 4096], fp8, isOutput=False)
    wbfd = nc.declare_dram_parameter("wbfd", [128, 32, 4096], bf16, isOutput=False)
    bias8 = nc.declare_dram_parameter("bias8", [1, 4096], bf16, isOutput=False)
    biasbf = nc.declare_dram_parameter("biasbf", [1, 4096], bf16, isOutput=False)
    wqc = nc.declare_dram_parameter("wqc", [128, 2, NB, 128], bf16, isOutput=False)
    wkc = nc.declare_dram_parameter("wkc", [128, 2, NB, 128], bf16, isOutput=False)
    wvc = nc.declare_dram_parameter("wvc", [128, 2, NB, 128], bf16, isOutput=False)
    fgw = nc.declare_dram_parameter("fgw", [128, 2 * BS], bf16, isOutput=False)
    fgb = nc.declare_dram_parameter("fgb", [1, 2 * BS], bf16, isOutput=False)
    hx_out = nc.declare_dram_parameter("hx_out", [BSH, NHID], f32, isOutput=True)
    cx_out = nc.declare_dram_parameter("cx_out", [BSH, NHID], f32, isOutput=True)
    mask_out = nc.declare_dram_parameter("mask_out", [BSH, NB], f32, isOutput=True)

    # ---- inline constants ----------------------------------------------
    hq_np = np.zeros((128, NB, 32), dtype=BF16)
    for d in range(128):
        for q in range(NB):
            hq_np[d, q, (d // 32) * 8 + q] = 1
    e32_np = np.zeros((32, NB, 128), dtype=BF16)
    for m in range(128):
        for q in range(NB):
            e32_np[(m // 32) * 8 + q, q, m] = 1
    # partition broadcaster: sel8[n', n, p] = (n' == n); a K=8 matmul with
    # lhsT=sel8[:, n, :] replicates row n of the rhs across 128 partitions
    sel8_np = np.zeros((8, NB, 128), dtype=BF16)
    for n in range(NB):
        sel8_np[n, n, :] = 1
    hqc = nc.inline_tensor(hq_np, "hqc")
    e32b = nc.inline_tensor(e32_np, "e32b")
    ones1c = nc.inline_tensor(np.ones((1, 128), dtype=BF16), "ones1c")
    sel8c = nc.inline_tensor(sel8_np, "sel8c")
    identc = nc.inline_tensor(np.eye(128, dtype=BF16), "identc")

    with tile.TileContext(nc) as tc:
        with tc.tile_pool(name="cp", bufs=1) as cp, \
             tc.tile_pool(name="pp", bufs=1) as pp:
            # fast-path inputs on sync (needed within ~5us)
            bias8_sb = cp.tile([1, 4096], bf16)
            nc.sync.dma_start(out=bias8_sb[:], in_=bias8[:])
            hxT8_sb = pp.tile([128, 16, BSH], fp8)
            hxTb_sb = pp.tile([128, 16, BSH], bf16)

            # constants and late inputs on gpsimd
            hq_sb = cp.tile([128, NB, 32], bf16)
            nc.gpsimd.dma_start(out=hq_sb[:], in_=hqc[:])
            e32_sb = cp.tile([32, NB, 128], bf16)
            nc.gpsimd.dma_start(out=e32_sb[:], in_=e32b[:])
            ones1_sb = cp.tile([1, 128], bf16)
            nc.gpsimd.dma_start(out=ones1_sb[:], in_=ones1c[:])
            fgw_sb = cp.tile([128, 2 * BS], bf16)
            nc.gpsimd.dma_start(out=fgw_sb[:], in_=fgw[:])
            fgb_sb = cp.tile([1, 2 * BS], bf16)
            nc.gpsimd.dma_start(out=fgb_sb[:], in_=fgb[:])
            wqc_sb = cp.tile([128, 2, NB, 128], bf16)
            nc.gpsimd.dma_start(out=wqc_sb[:], in_=wqc[:])
            wkc_sb = cp.tile([128, 2, NB, 128], bf16)
            nc.gpsimd.dma_start(out=wkc_sb[:], in_=wkc[:])
            wvc_sb = cp.tile([128, 2, NB, 128], bf16)
            nc.gpsimd.dma_start(out=wvc_sb[:], in_=wvc[:])
            cx_sb = [pp.tile([128, NHID], f32, tag=f"cx{bt}", name=f"cx{bt}")
                     for bt in range(2)]
            for bt in range(2):
                nc.gpsimd.dma_start(out=cx_sb[bt][:],
                                    in_=cx_bm[bt * 128:(bt + 1) * 128, :])
            hx_sb = [pp.tile([128, NHID], f32, tag=f"hx{bt}", name=f"hx{bt}")
                     for bt in range(2)]
            for bt in range(2):
                nc.gpsimd.dma_start(out=hx_sb[bt][:],
                                    in_=hx_bm[bt * 128:(bt + 1) * 128, :])

            sel8_sb = cp.tile([8, NB, 128], bf16)
            nc.gpsimd.dma_start(out=sel8_sb[:], in_=sel8c[:])
            ident_sb = cp.tile([128, 128], bf16)
            nc.gpsimd.dma_start(out=ident_sb[:], in_=identc[:])
            xt8_sb = pp.tile([128, 16, BSH], fp8)
            xtb_sb = pp.tile([128, 16, BSH], bf16)
            # hnew_sb holds h_new per group, overwritten in place by
            # d0 = h_new - hx once hnb/hnewT snapshots are taken
            hnew_sb = [pp.tile([128, NHID], f32, tag=f"hn{bt}", name=f"hn{bt}")
                       for bt in range(2)]
            hnewT_sb = pp.tile([128, 16, BSH], bf16)
            mask_sb = [pp.tile([128, NB], f32, tag=f"mk{bt}", name=f"mk{bt}")
                       for bt in range(2)]
            sig_sb = [pp.tile([128, NB], bf16, tag=f"sg{bt}", name=f"sg{bt}")
                      for bt in range(2)]
            qc_sb = pp.tile([128, NB, BSH], bf16)
            kc_sb = pp.tile([128, NB, BSH], bf16)
            vc_sb = pp.tile([128, NB, BSH], bf16)

            # ---- phase A (f32, mask-exact) -------------------------------
            with tc.tile_pool(name="pa", bufs=1) as pa, \
                 tc.tile_pool(name="pa2", bufs=2) as pa2, \
                 tc.tile_pool(name="paps", bufs=1, space="PSUM") as paps:
                # A inputs first on the scalar queue (arrive ~3us)
                inpT_sb = pa.tile([128, 8, BSH], f32)
                nc.sync.dma_start(out=inpT_sb[:], in_=inpT[:])
                wv1_sb = pa.tile([128, 8, BS], f32)
                nc.scalar.dma_start(out=wv1_sb[:], in_=wv1[:])
                wk1_sb = pa.tile([128, 8, DKI], f32)
                nc.sync.dma_start(out=wk1_sb[:], in_=wk1[:])
                wq_sb = pa.tile([128, 2, NB, DKI], f32)
                nc.scalar.dma_start(out=wq_sb[:], in_=wq[:])
                hxTf_sb = pa.tile([128, 16, BSH], f32)
                nc.scalar.dma_start(out=hxTf_sb[:], in_=hxT_f[:])
                nc.sync.dma_start(out=hxT8_sb[:], in_=hxT_8[:])
                nc.sync.dma_start(out=hxTb_sb[:], in_=hxT_b[:])
                biasbf_sb = cp.tile([1, 4096], bf16)
                nc.scalar.dma_start(out=biasbf_sb[:], in_=biasbf[:])

                # v1^T = wv1^T @ inp^T directly in feature-major (both halves)
                v1T_sb = pa.tile([128, 2, BSH], f32)
                for s in range(2):
                    v1T_ps = paps.tile([128, BSH], f32, tag="v1T")
                    for k in range(8):
                        nc.tensor.matmul(v1T_ps[:],
                                         wv1_sb[:, k, s * 128:(s + 1) * 128],
                                         inpT_sb[:, k, :],
                                         start=(k == 0), stop=(k == 7))
                    nc.vector.tensor_copy(v1T_sb[:, s, :], v1T_ps[:])

                sigT_sb = pa.tile([8, BSH], bf16)
                for bt in range(2):
                    bsl = slice(bt * 128, (bt + 1) * 128)
                    k1_ps = paps.tile([128, DKI], f32, tag="k1")
                    for k in range(8):
                        nc.tensor.matmul(k1_ps[:], inpT_sb[:, k, bsl],
                                         wk1_sb[:, k, :],
                                         start=(k == 0), stop=(k == 7))
                    k1s = pa2.tile([128, DKI], f32, tag="k1s")
                    nc.vector.tensor_copy(k1s[:], k1_ps[:])

                    q_ps = paps.tile([128, NB, DKI], f32, tag="q")
                    for n in range(NB):
                        for s in range(2):
                            nc.tensor.matmul(q_ps[:, n, :],
                                             hxTf_sb[:, 2 * n + s, bsl],
                                             wq_sb[:, s, n, :],
                                             start=(s == 0), stop=(s == 1))
                    prod = pa2.tile([128, NB, DKI], f32, tag="prod")
                    k1a = k1s[:]
                    k1bc = bass.AP(tensor=k1a.tensor, offset=k1a.offset,
                                   ap=[k1a.ap[0], [0, NB], k1a.ap[1]])
                    nc.vector.tensor_tensor(prod[:], q_ps[:], k1bc, OP.mult)
                    s1 = pa2.tile([128, NB], f32, tag="s1")
                    nc.vector.reduce_sum(s1[:], prod[:], axis=AX.X)
                    nc.scalar.activation(sig_sb[bt][:], s1[:], AF.Sigmoid,
                                         scale=0.125)

                    # top-4 mask (rank counts fused via accum_out)
                    cnt = pa2.tile([128, NB], f32, tag="cnt")
                    tmp = pa2.tile([128, NB], f32, tag="tmp")
                    for n in range(NB):
                        nc.vector.tensor_scalar(tmp[:], s1[:], s1[:, n:n + 1],
                                                0.0, OP.is_gt, OP.add,
                                                accum_out=cnt[:, n:n + 1])
                    nc.vector.tensor_single_scalar(mask_sb[bt][:], cnt[:], 4.0,
                                                   OP.is_lt)
                    nc.gpsimd.dma_start(out=mask_out[bsl, :], in_=mask_sb[bt][:])
                    # sig^T half for the partition broadcast below
                    sgt = paps.tile([8, 128], bf16, tag="sgt")
                    nc.tensor.transpose(sgt[:], sig_sb[bt][:], ident_sb[:])
                    nc.vector.tensor_copy(sigT_sb[:, bsl], sgt[:])

                # inp_flat^T = v1^T * broadcast(sig^T) per block, cast to
                # bf16 (f,g path) and fp8 (DoubleRow path)
                with tc.tile_pool(name="pasg", bufs=2, space="PSUM") as pasg:
                    for n in range(NB):
                        sgb = pasg.tile([128, BSH], f32, tag="sgb")
                        nc.tensor.matmul(sgb[:], sel8_sb[:, n, :], sigT_sb[:],
                                         start=True, stop=True)
                        for s in range(2):
                            nc.vector.tensor_tensor(xtb_sb[:, 2 * n + s, :],
                                                    v1T_sb[:, s, :], sgb[:],
                                                    OP.mult)
                            nc.gpsimd.tensor_copy(xt8_sb[:, 2 * n + s, :],
                                                  xtb_sb[:, 2 * n + s, :])

            # ---- phase B: LSTM groups, two per weight fetch --------------
            pair_order = list(range(8, 16)) + list(range(8))
            with tc.tile_pool(name="gps", bufs=1, space="PSUM") as gps, \
                 tc.tile_pool(name="pw", bufs=8) as pw, \
                 tc.tile_pool(name="pb2", bufs=2) as pb2:
                for gpair in range(4):
                    g_all = {}
                    for sub in range(2):
                        for bt in range(2):
                            g_all[sub, bt] = gps.tile(
                                [128, 4, BS], f32, tag=f"g{sub}{bt}",
                                name=f"g{sub}{bt}")
                    csl2 = slice(gpair * 1024, (gpair + 1) * 1024)
                    for sub in range(2):
                        gq = 2 * gpair + sub
                        csl = slice(gq * 512, (gq + 1) * 512)
                        for bt in range(2):
                            nc.tensor.matmul(g_all[sub, bt][:, 0:2, :],
                                             ones1_sb[:], bias8_sb[:, csl],
                                             start=True, stop=False)
                            nc.tensor.matmul(g_all[sub, bt][:, 2:4, :],
                                             ones1_sb[:], biasbf_sb[:, csl],
                                             start=True, stop=False)
                    for j in pair_order:
                        w8t = pw.tile([128, 2, 1024], fp8, tag="w8t")
                        nc.sync.dma_start(out=w8t[:],
                                          in_=w8d[:, 2 * j:2 * j + 2, csl2])
                        wbt = pw.tile([128, 2, 1024], bf16, tag="wbt")
                        nc.sync.dma_start(out=wbt[:],
                                          in_=wbfd[:, 2 * j:2 * j + 2, csl2])
                        st = (j == pair_order[-1])
                        for sub in range(2):
                            off = sub * 512
                            for bt in range(2):
                                bsl = slice(bt * 128, (bt + 1) * 128)
                                if j >= 8:
                                    t = 2 * (j - 8)
                                    lhs8 = [hxT8_sb[:, t + kk, bsl]
                                            for kk in range(2)]
                                    lhsb = [hxTb_sb[:, t + kk, bsl]
                                            for kk in range(2)]
                                else:
                                    lhs8 = [xt8_sb[:, 2 * j + kk, bsl]
                                            for kk in range(2)]
                                    lhsb = [xtb_sb[:, 2 * j + kk, bsl]
                                            for kk in range(2)]
                                for kk in range(2):
                                    nc.tensor.matmul(
                                        g_all[sub, bt][:, 0:2, :], lhs8[kk],
                                        w8t[:, kk, off:off + 512],
                                        start=False, stop=(st and kk == 1))
                                    nc.tensor.matmul(
                                        g_all[sub, bt][:, 2:4, :], lhsb[kk],
                                        wbt[:, kk, off:off + 512],
                                        start=False, stop=(st and kk == 1))
                    # ---- group tails ----------------------------------
                    # PSUM-freeing activation reads first for all four
                    # (sub, bt) so the next group-pair's matmuls unblock
                    # after ~3 ACT ops instead of a full tail chain.
                    acts = {}
                    for sub in range(2):
                        for bt in range(2):
                            sio = pb2.tile([128, 2, BS], f32, tag="sio",
                                           name="sio", bufs=4)
                            nc.scalar.activation(sio[:], g_all[sub, bt][:, 0:2, :],
                                                 AF.Sigmoid)
                            sigf = pb2.tile([128, BS], f32, tag="sigf",
                                            name="sigf", bufs=4)
                            nc.scalar.activation(sigf[:], g_all[sub, bt][:, 2, :],
                                                 AF.Sigmoid)
                            tang = pb2.tile([128, BS], f32, tag="tang",
                                            name="tang", bufs=4)
                            nc.scalar.activation(tang[:], g_all[sub, bt][:, 3, :],
                                                 AF.Tanh)
                            acts[sub, bt] = (sio, sigf, tang)
                    for sub in range(2):
                        gq = 2 * gpair + sub
                        sl = slice(gq * BS, (gq + 1) * BS)
                        for bt in range(2):
                            sio, sigf, tang = acts[sub, bt]
                            t1 = pb2.tile([128, BS], f32, tag="t1", name="t1")
                            nc.vector.tensor_tensor(t1[:], sigf[:],
                                                    cx_sb[bt][:, sl], OP.mult)
                            t2 = pb2.tile([128, BS], f32, tag="t2", name="t2")
                            nc.gpsimd.tensor_tensor(t2[:], sio[:, 0, :], tang[:],
                                                    OP.mult)
                            cnew = pb2.tile([128, BS], f32, tag="cnew",
                                            name="cnew")
                            nc.vector.tensor_tensor(cnew[:], t1[:], t2[:], OP.add)
                            t3 = pb2.tile([128, BS], f32, tag="t3", name="t3")
                            nc.scalar.activation(t3[:], cnew[:], AF.Tanh)
                            nc.vector.tensor_tensor(hnew_sb[bt][:, sl],
                                                    sio[:, 1, :], t3[:], OP.mult)
                            hnb = pb2.tile([128, BS], bf16, tag="hnb", name="hnb")
                            nc.vector.tensor_copy(hnb[:], hnew_sb[bt][:, sl])
                            dc = pb2.tile([128, BS], f32, tag="dc", name="dc")
                            nc.gpsimd.tensor_tensor(dc[:], cnew[:],
                                                    cx_sb[bt][:, sl], OP.subtract)
                            co = pb2.tile([128, BS], f32, tag="co", name="co")
                            nc.vector.scalar_tensor_tensor(
                                co[:], dc[:], mask_sb[bt][:, gq:gq + 1],
                                cx_sb[bt][:, sl], OP.mult, OP.add)
                            nc.gpsimd.dma_start(
                                out=cx_out[bt * 128:(bt + 1) * 128, sl],
                                in_=co[:])
                            for s in range(2):
                                nc.scalar.dma_start(
                                    out=hnewT_sb[:, 2 * gq + s,
                                                 bt * 128:(bt + 1) * 128],
                                    in_=hnb[:, s * 128:(s + 1) * 128],
                                    transpose=True)
                            # d0 = h_new - hx, in place (merge shortcut)
                            nc.gpsimd.tensor_tensor(hnew_sb[bt][:, sl],
                                                    hnew_sb[bt][:, sl],
                                                    hx_sb[bt][:, sl],
                                                    OP.subtract)

            # ============================ phase C ========================
            with tc.tile_pool(name="pc", bufs=1) as pc, \
                 tc.tile_pool(name="pctmp", bufs=3) as pctmp:
                at_sb = pc.tile([32, NB, BSH], bf16)
                coutb_sb = pc.tile([128, NB, BSH], bf16)
                with tc.tile_pool(name="psS", bufs=1, space="PSUM") as psS, \
                     tc.tile_pool(name="prjC", bufs=2, space="PSUM") as prj:
                    for n in range(NB):
                        ps = prj.tile([128, BSH], f32, tag="proj")
                        for s in range(2):
                            nc.tensor.matmul(ps[:], wkc_sb[:, s, n, :],
                                             hnewT_sb[:, 2 * n + s, :],
                                             start=(s == 0), stop=(s == 1))
                        nc.scalar.copy(kc_sb[:, n, :], ps[:])
                    S = psS.tile([32, NB, BSH], f32, tag="S", name="S")
                    for q in range(NB):
                        ps = prj.tile([128, BSH], f32, tag="proj")
                        for s in range(2):
                            nc.tensor.matmul(ps[:], wqc_sb[:, s, q, :],
                                             hnewT_sb[:, 2 * q + s, :],
                                             start=(s == 0), stop=(s == 1))
                        nc.scalar.copy(qc_sb[:, q, :], ps[:])
                        pr = pctmp.tile([128, NB, BSH], bf16, tag="pr", name="pr")
                        qa = qc_sb[:, q, :]
                        qbc = bass.AP(tensor=qa.tensor, offset=qa.offset,
                                      ap=[qa.ap[0], [0, NB], qa.ap[-1]])
                        nc.vector.tensor_tensor(pr[:], qbc, kc_sb[:], OP.mult)
                        for kp in range(4):
                            nc.tensor.matmul(S[:, 2 * kp:2 * kp + 2, :],
                                             hq_sb[:, q, :],
                                             pr[:, 2 * kp:2 * kp + 2, :],
                                             start=(q == 0), stop=(q == 7))
                    for n in range(NB):
                        ps = prj.tile([128, BSH], f32, tag="proj")
                        for s in range(2):
                            nc.tensor.matmul(ps[:], wvc_sb[:, s, n, :],
                                             hnewT_sb[:, 2 * n + s, :],
                                             start=(s == 0), stop=(s == 1))
                        nc.scalar.copy(vc_sb[:, n, :], ps[:])
                    ex = pc.tile([32, NB, BSH], bf16, tag="ex", name="ex")
                    nc.scalar.activation(ex[:], S[:], AF.Exp,
                                         scale=float(1.0 / np.sqrt(32.0)))
                    # denominator by bf16 tree adds (contiguous slices)
                    e1 = pctmp.tile([32, 4, BSH], bf16, tag="e1", name="e1")
                    nc.vector.tensor_tensor(e1[:], ex[:, 0:4, :], ex[:, 4:8, :],
                                            OP.add)
                    e2 = pctmp.tile([32, 2, BSH], bf16, tag="e2", name="e2")
                    nc.vector.tensor_tensor(e2[:], e1[:, 0:2, :], e1[:, 2:4, :],
                                            OP.add)
                    denom = pctmp.tile([32, BSH], f32, tag="denom", name="denom")
                    nc.vector.tensor_tensor(denom[:], e2[:, 0, :], e2[:, 1, :],
                                            OP.add)
                    recip = pctmp.tile([32, BSH], f32, tag="recip", name="recip")
                    nc.vector.reciprocal(recip[:], denom[:])
                    ra = recip[:]
                    rbc = bass.AP(tensor=ra.tensor, offset=ra.offset,
                                  ap=[ra.ap[0], [0, NB], ra.ap[-1]])
                    nc.vector.tensor_tensor(at_sb[:], ex[:], rbc, OP.mult)

                with tc.tile_pool(name="psU", bufs=1, space="PSUM") as psU, \
                     tc.tile_pool(name="psOG", bufs=4, space="PSUM") as psOG:
                    for q in range(NB):
                        U = psU.tile([128, NB, BSH], f32, tag="U", name="U")
                        for kp in range(4):
                            nc.tensor.matmul(U[:, 2 * kp:2 * kp + 2, :],
                                             e32_sb[:, q, :],
                                             at_sb[:, 2 * kp:2 * kp + 2, :],
                                             start=True, stop=True)
                        prods = pctmp.tile([128, NB, BSH], bf16, tag="prods")
                        nc.vector.tensor_tensor(prods[:], U[:], vc_sb[:], OP.mult)
                        tr1 = pctmp.tile([128, 4, BSH], bf16, tag="tr1",
                                         name="tr1")
                        nc.vector.tensor_tensor(tr1[:], prods[:, 0:4, :],
                                                prods[:, 4:8, :], OP.add)
                        tr2 = pctmp.tile([128, 2, BSH], bf16, tag="tr2",
                                         name="tr2")
                        nc.vector.tensor_tensor(tr2[:], tr1[:, 0:2, :],
                                                tr1[:, 2:4, :], OP.add)
                        nc.vector.tensor_tensor(coutb_sb[:, q, :], tr2[:, 0, :],
                                                tr2[:, 1, :], OP.add)
                        # gated residual + masked merge for this block
                        for bt in range(2):
                            csl = coutb_sb[:, q, bt * 128:(bt + 1) * 128]
                            og = psOG.tile([128, 2 * BS], f32, tag="og",
                                           name="og")
                            nc.tensor.matmul(og[:], csl, fgw_sb[:],
                                             start=True, stop=False)
                            nc.tensor.matmul(og[:], ones1_sb[:], fgb_sb[:],
                                             start=False, stop=True)
                            tano = pctmp.tile([128, BS], f32, tag="tano",
                                              name="tano")
                            nc.scalar.activation(tano[:], og[:, 0:BS], AF.Tanh)
                            sg = pctmp.tile([128, BS], f32, tag="sgx", name="sgx")
                            nc.scalar.activation(sg[:], og[:, BS:2 * BS],
                                                 AF.Sigmoid)
                            hatt = pctmp.tile([128, BS], f32, tag="hatt",
                                              name="hatt")
                            nc.vector.tensor_tensor(hatt[:], sg[:], tano[:],
                                                    OP.mult)
                            qsl = slice(q * BS, (q + 1) * BS)
                            # dh = d0 + hatt ; ho = mask*dh + hx
                            dh = pctmp.tile([128, BS], f32, tag="dhq", name="dhq")
                            nc.gpsimd.tensor_tensor(dh[:], hnew_sb[bt][:, qsl],
                                                    hatt[:], OP.add)
                            ho = pctmp.tile([128, BS], f32, tag="hoq", name="hoq",
                                            bufs=4)
                            nc.vector.scalar_tensor_tensor(ho[:], dh[:],
                                                           mask_sb[bt][:, q:q + 1],
                                                           hx_sb[bt][:, qsl],
                                                           OP.mult, OP.add)
                            nc.gpsimd.dma_start(
                                out=hx_out[bt * 128:(bt + 1) * 128, qsl],
                                in_=ho[:])

    _install_bir_fix(nc)
    return nc


# ---------------------------------------------------------------------------
# Host wrapper
# ---------------------------------------------------------------------------

def kernel(inp, hx, cx, wq_inp, wk_inp, wv_inp, w_ih, w_hh, b_ih, b_hh,
           wq_c, wk_c, wv_c, fc_w, fc_b, gate_w, gate_b, step=None):
    global last_exec_time_ns, last_results

    inp = np.asarray(inp, np.float32)
    hx = np.asarray(hx, np.float32)
    cx = np.asarray(cx, np.float32)
    wq_inp = np.asarray(wq_inp, np.float32)
    wk_inp = np.asarray(wk_inp, np.float32)
    wv_inp = np.asarray(wv_inp, np.float32)
    w_ih = np.asarray(w_ih, np.float32)
    w_hh = np.asarray(w_hh, np.float32)
    b_ih = np.asarray(b_ih, np.float32)
    b_hh = np.asarray(b_hh, np.float32)
    wq_c = np.asarray(wq_c, np.float32)
    wk_c = np.asarray(wk_c, np.float32)
    wv_c = np.asarray(wv_c, np.float32)
    fc_w = np.asarray(fc_w, np.float32)
    fc_b = np.asarray(fc_b, np.float32)
    gate_w = np.asarray(gate_w, np.float32)
    gate_b = np.asarray(gate_b, np.float32)

    if "nc" not in _CACHE:
        _CACHE["nc"] = _build()
    nc = _CACHE["nc"]

    # column permutations: per 256-wide hidden group g the fp8 panel holds
    # [i|o], the bf16 panel [f|g]  (torch gate order i,f,g,o)
    wcat = np.concatenate([w_ih.T, w_hh.T], axis=0)     # (4096, 8192)
    bias = (b_ih + b_hh)
    perm8 = np.concatenate([np.concatenate([
        np.arange(0 * NHID + g * BS, 0 * NHID + (g + 1) * BS),
        np.arange(3 * NHID + g * BS, 3 * NHID + (g + 1) * BS)])
        for g in range(8)])
    permbf = np.concatenate([np.concatenate([
        np.arange(1 * NHID + g * BS, 1 * NHID + (g + 1) * BS),
        np.arange(2 * NHID + g * BS, 2 * NHID + (g + 1) * BS)])
        for g in range(8)])
    w8_np = wcat[:, perm8].astype(E4)                   # (4096, 4096)
    wbf_np = wcat[:, permbf].astype(BF16)
    w8d = np.ascontiguousarray(w8_np.reshape(32, 128, 4096).transpose(1, 0, 2))
    wbfd = np.ascontiguousarray(wbf_np.reshape(32, 128, 4096).transpose(1, 0, 2))

    shared = {
        "wq": np.ascontiguousarray(
            wq_inp.reshape(NB, 2, 128, DKI).transpose(2, 1, 0, 3)),
        "wk1": np.ascontiguousarray(
            wk_inp[1].reshape(8, 128, DKI).transpose(1, 0, 2)),
        "wv1": np.ascontiguousarray(
            wv_inp[1].reshape(8, 128, BS).transpose(1, 0, 2)),
        "w8d": w8d,
        "wbfd": wbfd,
        "bias8": bias[perm8].astype(BF16).reshape(1, 4096),
        "biasbf": bias[permbf].astype(BF16).reshape(1, 4096),
        "wqc": np.ascontiguousarray(
            wq_c.astype(BF16).reshape(NB, 2, 128, 128).transpose(2, 1, 0, 3)),
        "wkc": np.ascontiguousarray(
            wk_c.astype(BF16).reshape(NB, 2, 128, 128).transpose(2, 1, 0, 3)),
        "wvc": np.ascontiguousarray(
            wv_c.astype(BF16).reshape(NB, 2, 128, 128).transpose(2, 1, 0, 3)),
        "fgw": np.ascontiguousarray(
            np.concatenate([fc_w, gate_w], axis=1)).astype(BF16),
        "fgb": np.concatenate([fc_b, gate_b]).astype(BF16).reshape(1, 2 * BS),
    }

    in_maps = []
    for c in range(NCORES):
        rs = slice(c * BSH, (c + 1) * BSH)
        inpT_c = inp[rs].T.reshape(8, 128, BSH).transpose(1, 0, 2)
        hxT = hx[rs].T.reshape(16, 128, BSH).transpose(1, 0, 2)
        m = {
            "inpT": np.ascontiguousarray(inpT_c),
            "hxT_f": np.ascontiguousarray(hxT),
            "hxT_b": np.ascontiguousarray(hxT.astype(BF16)),
            "hxT_8": np.ascontiguousarray(hxT.astype(E4)),
            "hx_bm": np.ascontiguousarray(hx[rs]),
            "cx_bm": np.ascontiguousarray(cx[rs]),
        }
        m.update(shared)
        in_maps.append(m)

    from concourse.bass_utils import run_bass_kernel_spmd
    trace = bool(int(os.environ.get("BASS_KTRACE", "0")))
    res = run_bass_kernel_spmd(nc, in_maps, list(range(NCORES)), trace=trace)
    last_exec_time_ns = res.exec_time_ns
    last_results = res

    hx_full = np.empty((B, NHID), np.float32)
    cx_full = np.empty((B, NHID), np.float32)
    mask_full = np.empty((B, NHID), np.float32)
    for c in range(NCORES):
        rs = slice(c * BSH, (c + 1) * BSH)
        hx_full[rs] = res.results[c]["hx_out"]
        cx_full[rs] = res.results[c]["cx_out"]
        mask_full[rs] = np.repeat(res.results[c]["mask_out"], BS, axis=1)
    return hx_full, cx_full, mask_full
